# revision 32
# baseline (speedup 1.0000x reference)
"""Multi-head self-attention (B=8, S=1024, D=1024, H=16) on 8 trn2 cores.

Sharding: pure data-parallel over batch (B=8 -> 1 batch per core, no
collectives). Per-core kernel computes the full attention for one batch.

v2: all matmul operands in bf16 (host-converted; fp32 PSUM accumulation,
rel err ~2.5e-3 vs the 2e-2 gate) and a software-pipelined emission order
that hides the softmax exp (ScalarE, ~131us total, no 16-bit speedup on
ACT) under the tensor-engine work:

  A: QT/KT projections (transposed layout, head dim on partitions),
     by-c psum8 structure, bf16 in/out.
  S: scores+exp for the first LAG units emitted right after A so the
     scalar engine starts while V is still projecting.
  C: V projection by-s-strip (1 psum slot per strip) from SBUF-resident
     xvT/wvT so it coexists with the early score units; V stored with a
     65th ones column per head (attn@V also yields the softmax sum).
  unit loop (16 units = 8 head-pairs x 2 q-chunks): attn@V accumulation,
     one wide reciprocal of the sumexp row (DVE), scores+exp for unit
     i+LAG (keeps PE busy while the reciprocal runs), PE rank-1
     broadcast of the reciprocals, normalize, odd head DMA-shifted to
     partitions 64..127 of OT.
  E: output projection from bf16 OT, fp32 out.

PSUM is split into two 2-slot pools (av/bc in A, score tiles in B) so
the strict round-robin slot rotation never makes a PE instruction wait
on a slot whose free depends on a later PE instruction (FIFO inversion
deadlock).
"""

import sys

for _p in ("/opt/trn_rl_repo", "/root/.axon_site/_ro/trn_rl_repo"):
    if _p not in sys.path:
        sys.path.append(_p)

import numpy as np
import ml_dtypes

import concourse.bass as bass
import concourse.mybir as mybir
import concourse.tile as tile
from concourse import bacc
from concourse.bass_utils import run_bass_kernel_spmd

F32 = mybir.dt.float32
F32R = mybir.dt.float32r
BF16 = mybir.dt.bfloat16
NPBF16 = ml_dtypes.bfloat16
EXP = mybir.ActivationFunctionType.Exp

S = 1024   # sequence length
D = 1024   # model dim
H = 16     # heads
DK = 64    # head dim
P = 128    # partitions
QC = 512   # q-chunk (psum bank free size in fp32)
NT = D // P   # 8 e-tiles / d-tiles / s-tiles
NB = 8     # batches == cores

VW = DK + 1  # 65: V columns per head incl. ones column
LAG = 3      # score/exp units emitted ahead of their attn@V consumer


def _emit(tc, io, phases="ASCVE"):
    nc = tc.nc

    const = tc.alloc_tile_pool(name="const", bufs=1)
    persist = tc.alloc_tile_pool(name="persist", bufs=1)
    stream = tc.alloc_tile_pool(name="stream", bufs=1)

    # --- constants -------------------------------------------------------
    # All const DMAs go through the Activation engine's DGE so the SP DGE can
    # start streaming phase-A x/w tiles immediately (HWDGE descriptor
    # processing is ~625ns each and serializes per engine queue).
    # per-partition bias columns: column t holds bias[t*128:(t+1)*128]
    bq_sb = const.tile([P, NT], F32, tag="bq", name="bq_sb")
    nc.scalar.dma_start(bq_sb[:], io["bqs"].ap().rearrange("(t p) -> p t", p=P))
    bk_sb = const.tile([P, NT], F32, tag="bk", name="bk_sb")
    nc.scalar.dma_start(bk_sb[:], io["bk"].ap().rearrange("(t p) -> p t", p=P))
    # row-broadcast bias tiles [128, D] for biases added along the free dim
    bv_bc = const.tile([P, D], F32, tag="bv", name="bv_bc")
    nc.scalar.dma_start(
        bv_bc[:].unsqueeze(1), io["bv"].ap().unsqueeze(0).partition_broadcast(P)
    )
    bo_bc = const.tile([P, D], F32, tag="bo", name="bo_bc")
    nc.scalar.dma_start(
        bo_bc[:].unsqueeze(1), io["bo"].ap().unsqueeze(0).partition_broadcast(P)
    )


    # --- persistent SBUF tensors (all bf16) ------------------------------
    QT = [persist.tile([P, S], BF16, tag=f"qt{t}", name=f"qt{t}") for t in range(NT)]
    KT = [persist.tile([P, S], BF16, tag=f"kt{t}", name=f"kt{t}") for t in range(NT)]
    # V with a ones column appended per head: [s, 16*65]
    V = [persist.tile([P, H * VW], F32R, tag=f"v{t}", name=f"v{t}") for t in range(NT)]
    OT = [persist.tile([P, S], BF16, tag=f"ot{t}", name=f"ot{t}") for t in range(NT)]
    # SBUF-resident wvT blocks for the by-strip V projection (xv slices
    # are streamed per strip; phase C hides under the exp-bound stretch)
    WV = [persist.tile([P, D], BF16, tag=f"wv{t}", name=f"wv{t}") for t in range(NT)]

    # ones columns of V (column 64 of each head's 65-wide group)
    for st in range(NT):
        v_view = V[st][:].rearrange("p (h k) -> p h k", k=VW)
        nc.scalar.dma_start(
            v_view[:, :, DK:VW].unsqueeze(1),
            io["onesw"].ap().unsqueeze(1).unsqueeze(0).partition_broadcast(P),
        )

    # Two PSUM pools, 2 slots x [128, 1024] (2 banks) each = all 8 banks.
    # Pool A: av/bc tiles in the unit loop; pool B: score tiles. Projections
    # draw 2 slots from each.
    poolA = tc.alloc_tile_pool(name="poolA", bufs=2, space="PSUM")
    poolB = tc.alloc_tile_pool(name="poolB", bufs=2, space="PSUM")

    def psum8(pfx):
        pairs = [
            (poolA if s < 2 else poolB).tile(
                [P, 2 * QC], F32, tag="u", name=f"{pfx}_{s}"
            )
            for s in range(NT // 2)
        ]
        return [pairs[t // 2][:, (t % 2) * QC : (t % 2 + 1) * QC] for t in range(NT)]

    # --- phase A: QT / KT projections (transposed, bias per-partition) ---
    for dst, wname, xname, bias in () if "A" not in phases else (
        (QT, "wqT", "xqT", bq_sb),
        (KT, "wkT", "xkT", bk_sb),
    ):
        w_ap = io[wname].ap()
        x_ap = io[xname].ap()
        wts = []  # w blocks stay resident across both c-chunks (loaded once)
        xt0 = None
        for c in range(2):
            ps = psum8(f"ps_{wname}{c}")
            for d in range(NT):
                if "a" in phases and xt0 is not None:
                    xt = xt0
                    wt = wts[0]
                else:
                    xt = stream.tile(
                        [P, QC], BF16, tag="xc", bufs=3, name=f"x_{xname}{c}_{d}"
                    )
                    nc.sync.dma_start(
                        xt[:], x_ap[d * P : (d + 1) * P, c * QC : (c + 1) * QC]
                    )
                    if xt0 is None:
                        xt0 = xt
                if "a" not in phases:
                    if c == 0:
                        wt = stream.tile(
                            [P, D], BF16, tag="big", bufs=8, name=f"w_{wname}_{d}"
                        )
                        nc.sync.dma_start(wt[:], w_ap[d * P : (d + 1) * P, :])
                        wts.append(wt)
                    else:
                        wt = wts[d]
                elif not wts:
                    wt = stream.tile(
                        [P, D], BF16, tag="big", bufs=8, name=f"w_{wname}_0"
                    )
                    nc.sync.dma_start(wt[:], w_ap[0:P, :])
                    wts.append(wt)
                for t in range(NT):
                    nc.tensor.matmul(
                        ps[t][:],
                        lhsT=wt[:, t * P : (t + 1) * P],
                        rhs=xt[:],
                        start=(d == 0),
                        stop=(d == NT - 1),
                    )
            for t in range(NT):
                nc.vector.tensor_scalar_add(
                    dst[t][:, c * QC : (c + 1) * QC], ps[t][:], bias[:, t : t + 1]
                )

    # prefetch the V-projection weights (used ~25us later; DMA has slack)
    xv_ap = io["xvT"].ap()
    if "C" in phases:
        wv_ap = io["wvT"].ap()
        for d in range(NT):
            nc.sync.dma_start(WV[d][:], wv_ap[d * P : (d + 1) * P, :])

    # --- scores + exp for one unit (p, qi) -------------------------------
    dsb = tc.alloc_tile_pool(name="dsb", bufs=1)
    ats = {}

    def emit_scores(p, qi):
        qs = slice(qi * QC, (qi + 1) * QC)
        groups = []
        for g in range(4):  # groups of 2 k-blocks -> [128, 1024] psum
            sce = poolB.tile([P, 2 * QC], F32, tag="u", name=f"sce{p}_{qi}_{g}")
            sco = poolB.tile([P, 2 * QC], F32, tag="u", name=f"sco{p}_{qi}_{g}")
            for j in range(2):
                kb = 2 * g + j
                ksl = slice(kb * P, (kb + 1) * P)
                nc.tensor.matmul(
                    sce[:, j * QC : (j + 1) * QC],
                    lhsT=KT[p][0:64, ksl],
                    rhs=QT[p][0:64, qs],
                    start=True,
                    stop=True,
                )
                nc.tensor.matmul(
                    sco[:, j * QC : (j + 1) * QC],
                    lhsT=KT[p][64:128, ksl],
                    rhs=QT[p][64:128, qs],
                    start=True,
                    stop=True,
                    tile_position=(64, 0),
                )
            at_dt = F32R
            at_bufs = 14
            ae = dsb.tile([P, 2 * QC], at_dt, tag="at", bufs=at_bufs,
                          name=f"ae{p}_{qi}_{g}")
            nc.scalar.activation(ae[:], sce[:], EXP)
            ao = dsb.tile([P, 2 * QC], at_dt, tag="at", bufs=at_bufs,
                          name=f"ao{p}_{qi}_{g}")
            nc.scalar.activation(ao[:], sco[:], EXP)
            groups.append((ae, ao))
        ats[(p, qi)] = groups

    units = [(p, qi) for p in range(NT) for qi in range(2)]
    if "S" in phases and "V" not in phases:
        for p, qi in units:
            emit_scores(p, qi)
    if "S" in phases and "V" in phases:
        for p, qi in units[:LAG]:
            emit_scores(p, qi)

    # --- phase C: V projection by s-strip (1 pool-A slot per strip) ------
    # scores for unit LAG are emitted mid-C so the scalar engine stays fed
    # with exp work through the end of C (its 3 early units only cover ~25us
    # of the ~27us projection).
    for st in range(NT) if "C" in phases else ():
        if st == 4 and "S" in phases and "V" in phases:
            emit_scores(*units[LAG])
        vp = poolA.tile([P, 2 * QC], F32, tag="u", name=f"vps{st}")
        for d in range(NT):
            xvs = stream.tile([P, P], BF16, tag="xvs", bufs=4, name=f"xv{st}_{d}")
            nc.gpsimd.dma_start(
                xvs[:], xv_ap[d * P : (d + 1) * P, st * P : (st + 1) * P]
            )
            for c in range(2):
                nc.tensor.matmul(
                    vp[:, c * QC : (c + 1) * QC],
                    lhsT=xvs[:],
                    rhs=WV[d][:, c * QC : (c + 1) * QC],
                    start=(d == 0),
                    stop=(d == NT - 1),
                )
        v_out = V[st][:].rearrange("p (h k) -> p h k", k=VW)[:, :, 0:DK]
        ps_v = vp[:].rearrange("p (h k) -> p h k", k=DK)
        bv_v = bv_bc[:].rearrange("p (h k) -> p h k", k=DK)
        nc.vector.tensor_add(v_out, ps_v, bv_v)

    # --- unit loop: attn@V, normalize; scores for unit i+LAG in between --
    for i, (p, qi) in enumerate(units) if "V" in phases else ():
        he, ho = 2 * p, 2 * p + 1
        qs = slice(qi * QC, (qi + 1) * QC)
        groups = ats.pop((p, qi))

        av = poolA.tile([P, 2 * QC], F32, tag="u", name=f"av{p}_{qi}")
        ave = av[:, 0:QC]
        avo = av[:, QC : 2 * QC]
        for g in range(4):
            ae, ao = groups[g]
            for j in range(2):
                kb = 2 * g + j
                nc.tensor.matmul(
                    ave[0:VW, :],
                    lhsT=V[kb][:, he * VW : (he + 1) * VW],
                    rhs=ae[:, j * QC : (j + 1) * QC],
                    start=(kb == 0),
                    stop=(kb == NT - 1),
                )
                nc.tensor.matmul(
                    avo[0:VW, :],
                    lhsT=V[kb][:, ho * VW : (ho + 1) * VW],
                    rhs=ao[:, j * QC : (j + 1) * QC],
                    start=(kb == 0),
                    stop=(kb == NT - 1),
                )

        # one wide reciprocal of the sumexp row (partition 64, both parities)
        rec = dsb.tile([P, 2 * QC], F32, tag="rec", bufs=2, name=f"rec{p}_{qi}")
        nc.vector.reciprocal(rec[64:65, :], av[64:65, :])

        # keep PE busy with the next unit's scores while DVE runs reciprocal
        # (units 0..LAG were emitted before/during C, so the loop emits i+LAG+1)
        if i + LAG + 1 < len(units):
            emit_scores(*units[i + LAG + 1])

        # broadcast the reciprocal row down to 64 partitions via a DRAM
        # bounce (SBUF->DRAM->SBUF partition-broadcast load, the proven
        # bias-load pattern; SP DGE is idle here and this keeps PE/PSUM out
        # of the normalize chain)
        scr = io["recscr"].ap()[i % 2]
        nc.sync.dma_start(scr, rec[64:65, :])
        rbc = dsb.tile([DK, 2 * QC], F32, tag="rbc", bufs=2, name=f"rbc{p}_{qi}")
        nc.sync.dma_start(
            rbc[:].unsqueeze(1), scr.unsqueeze(0).partition_broadcast(DK)
        )
        nc.vector.tensor_mul(OT[p][0:64, qs], ave[0:64, :], rbc[:, 0:QC])
        tmpo = dsb.tile([64, QC], BF16, tag="tmp", bufs=2, name=f"tmpo{p}_{qi}")
        nc.vector.tensor_mul(tmpo[:], avo[0:64, :], rbc[:, QC : 2 * QC])
        nc.sync.dma_start(OT[p][64:128, qs], tmpo[:])

    # --- phase E: output projection out[s, f] = OT.T @ woT + bo ----------
    out_ap = io["out"].ap()
    wo_ap = io["woT"].ap()
    if "E" not in phases:
        # bench-only: drain something comparable to E's output traffic
        srcs = OT if "V" in phases else QT
        for t in range(NT):
            nc.sync.dma_start(out_ap[t * P : (t + 1) * P, :], srcs[t][:, 0:S])
    for c in range(2) if "E" in phases else ():
        fs = slice(c * QC, (c + 1) * QC)
        ps = psum8(f"ps_o{c}")
        for e in range(NT):
            wt = stream.tile([P, QC], BF16, tag="xc", bufs=3, name=f"w_o{c}_{e}")
            nc.sync.dma_start(wt[:], wo_ap[e * P : (e + 1) * P, fs])
            for st in range(NT):
                nc.tensor.matmul(
                    ps[st][:],
                    lhsT=OT[e][:, st * P : (st + 1) * P],
                    rhs=wt[:],
                    start=(e == 0),
                    stop=(e == NT - 1),
                )
        for st in range(NT):
            ob = stream.tile([P, QC], BF16, tag="ob", bufs=2, name=f"ob{c}_{st}")
            nc.vector.tensor_add(ob[:], ps[st][:], bo_bc[:, fs])
            nc.sync.dma_start(out_ap[st * P : (st + 1) * P, fs], ob[:])

    dsb.release()
    poolB.release()
    poolA.release()
    stream.release()
    persist.release()
    const.release()


def build_nc(repeats=1, phases="ASCVE"):
    nc = bacc.Bacc(
        "TRN2",
        target_bir_lowering=False,
        debug=False,
        enable_asserts=False,
        num_devices=NB,
    )
    io = {}
    for name in ("xqT", "xkT", "xvT"):
        io[name] = nc.dram_tensor(name, [D, S], BF16, kind="ExternalInput")
    for name in ("wqT", "wkT", "wvT", "woT"):
        io[name] = nc.dram_tensor(name, [D, D], BF16, kind="ExternalInput")
    for name in ("bqs", "bk", "bv", "bo"):
        io[name] = nc.dram_tensor(name, [D], F32, kind="ExternalInput")
    io["onesw"] = nc.dram_tensor("onesw", [H], F32R, kind="ExternalInput")
    io["out"] = nc.dram_tensor("out", [S, D], BF16, kind="ExternalOutput")
    io["recscr"] = nc.dram_tensor("recscr", [2, 2 * QC], F32, kind="Internal")

    with tile.TileContext(nc) as tc:
        for _ in range(repeats):
            _emit(tc, io, phases)
    nc.compile()
    return nc


_CACHE = {}


def get_nc():
    if "nc" not in _CACHE:
        _CACHE["nc"] = build_nc()
    return _CACHE["nc"]


def make_in_maps(query, key, value, wq, bq, wk, bk, wv, bv, wo, bo):
    f = np.float32
    # fold the 1/sqrt(DK) score scaling into the Q projection (exact: 1/8)
    wqT = (np.asarray(wq, f).T * f(0.125)).astype(NPBF16)
    bqs = np.asarray(bq, f) * f(0.125)
    wkT = np.asarray(wk, f).T.astype(NPBF16)
    wvT = np.asarray(wv, f).T.astype(NPBF16)
    woT = np.asarray(wo, f).T.astype(NPBF16)
    common = {
        "wqT": np.ascontiguousarray(wqT),
        "wkT": np.ascontiguousarray(wkT),
        "wvT": np.ascontiguousarray(wvT),
        "woT": np.ascontiguousarray(woT),
        "bqs": np.ascontiguousarray(bqs),
        "bk": np.ascontiguousarray(np.asarray(bk, f)),
        "bv": np.ascontiguousarray(np.asarray(bv, f)),
        "bo": np.ascontiguousarray(np.asarray(bo, f)),
        "onesw": np.ones(H, np.float32),
    }
    q = np.asarray(query, f)
    k = np.asarray(key, f)
    v = np.asarray(value, f)
    in_maps = []
    for b in range(NB):
        in_maps.append(
            {
                "xqT": np.ascontiguousarray(q[b].T.astype(NPBF16)),
                "xkT": np.ascontiguousarray(k[b].T.astype(NPBF16)),
                "xvT": np.ascontiguousarray(v[b].T.astype(NPBF16)),
                **common,
            }
        )
    return in_maps


def kernel(
    query,
    key,
    value,
    inputs_attn_mask=None,  # all-ones per spec; masking is a no-op
    wq=None, bq=None, wk=None, bk=None, wv=None, bv=None, wo=None, bo=None,
    **_extra,
):
    nc = get_nc()
    in_maps = make_in_maps(query, key, value, wq, bq, wk, bk, wv, bv, wo, bo)
    res = run_bass_kernel_spmd(nc, in_maps, core_ids=list(range(NB)))
    out = np.stack(
        [np.asarray(res.results[b]["out"]).astype(np.float32) for b in range(NB)],
        axis=0,
    )
    return out


# revision 33
# speedup vs baseline: 1.0119x; 1.0119x over previous
"""Multi-head self-attention (B=8, S=1024, D=1024, H=16) on 8 trn2 cores.

Sharding: pure data-parallel over batch (B=8 -> 1 batch per core, no
collectives). Per-core kernel computes the full attention for one batch.

v2: all matmul operands in bf16 (host-converted; fp32 PSUM accumulation,
rel err ~2.5e-3 vs the 2e-2 gate) and a software-pipelined emission order
that hides the softmax exp (ScalarE, ~131us total, no 16-bit speedup on
ACT) under the tensor-engine work:

  A: QT/KT projections (transposed layout, head dim on partitions),
     by-c psum8 structure, bf16 in/out.
  S: scores+exp for the first LAG units emitted right after A so the
     scalar engine starts while V is still projecting.
  C: V projection by-s-strip (1 psum slot per strip) from SBUF-resident
     xvT/wvT so it coexists with the early score units; V stored with a
     65th ones column per head (attn@V also yields the softmax sum).
  unit loop (16 units = 8 head-pairs x 2 q-chunks): attn@V accumulation,
     one wide reciprocal of the sumexp row (DVE), scores+exp for unit
     i+LAG (keeps PE busy while the reciprocal runs), PE rank-1
     broadcast of the reciprocals, normalize, odd head DMA-shifted to
     partitions 64..127 of OT.
  E: output projection from bf16 OT, fp32 out.

PSUM is split into two 2-slot pools (av/bc in A, score tiles in B) so
the strict round-robin slot rotation never makes a PE instruction wait
on a slot whose free depends on a later PE instruction (FIFO inversion
deadlock).
"""

import sys

for _p in ("/opt/trn_rl_repo", "/root/.axon_site/_ro/trn_rl_repo"):
    if _p not in sys.path:
        sys.path.append(_p)

import numpy as np
import ml_dtypes

import concourse.bass as bass
import concourse.mybir as mybir
import concourse.tile as tile
from concourse import bacc
from concourse.bass_utils import run_bass_kernel_spmd

F32 = mybir.dt.float32
F32R = mybir.dt.float32r
BF16 = mybir.dt.bfloat16
NPBF16 = ml_dtypes.bfloat16
EXP = mybir.ActivationFunctionType.Exp

S = 1024   # sequence length
D = 1024   # model dim
H = 16     # heads
DK = 64    # head dim
P = 128    # partitions
QC = 512   # q-chunk (psum bank free size in fp32)
NT = D // P   # 8 e-tiles / d-tiles / s-tiles
NB = 8     # batches == cores

VW = DK + 1  # 65: V columns per head incl. ones column
LAG = 3      # score/exp units emitted ahead of their attn@V consumer


def _emit(tc, io, phases="ASCVE"):
    nc = tc.nc

    const = tc.alloc_tile_pool(name="const", bufs=1)
    persist = tc.alloc_tile_pool(name="persist", bufs=1)
    stream = tc.alloc_tile_pool(name="stream", bufs=1)

    # --- constants -------------------------------------------------------
    # All const DMAs go through the Activation engine's DGE so the SP DGE can
    # start streaming phase-A x/w tiles immediately (HWDGE descriptor
    # processing is ~625ns each and serializes per engine queue).
    # per-partition bias columns: column t holds bias[t*128:(t+1)*128]
    bq_sb = const.tile([P, NT], F32, tag="bq", name="bq_sb")
    nc.scalar.dma_start(bq_sb[:], io["bqs"].ap().rearrange("(t p) -> p t", p=P))
    bk_sb = const.tile([P, NT], F32, tag="bk", name="bk_sb")
    nc.scalar.dma_start(bk_sb[:], io["bk"].ap().rearrange("(t p) -> p t", p=P))
    # row-broadcast bias tiles [128, D] for biases added along the free dim
    bv_bc = const.tile([P, D], F32, tag="bv", name="bv_bc")
    nc.scalar.dma_start(
        bv_bc[:].unsqueeze(1), io["bv"].ap().unsqueeze(0).partition_broadcast(P)
    )
    bo_bc = const.tile([P, D], F32, tag="bo", name="bo_bc")
    nc.scalar.dma_start(
        bo_bc[:].unsqueeze(1), io["bo"].ap().unsqueeze(0).partition_broadcast(P)
    )


    # --- persistent SBUF tensors (all bf16) ------------------------------
    QT = [persist.tile([P, S], BF16, tag=f"qt{t}", name=f"qt{t}") for t in range(NT)]
    KT = [persist.tile([P, S], BF16, tag=f"kt{t}", name=f"kt{t}") for t in range(NT)]
    # V with a ones column appended per head: [s, 16*65]
    V = [persist.tile([P, H * VW], BF16, tag=f"v{t}", name=f"v{t}") for t in range(NT)]
    OT = [persist.tile([P, S], BF16, tag=f"ot{t}", name=f"ot{t}") for t in range(NT)]
    # SBUF-resident xvT / wvT blocks for the by-strip V projection
    XV = [persist.tile([P, S], BF16, tag=f"xv{t}", name=f"xv{t}") for t in range(NT)]
    WV = [persist.tile([P, D], BF16, tag=f"wv{t}", name=f"wv{t}") for t in range(NT)]

    # ones columns of V (column 64 of each head's 65-wide group)
    for st in range(NT):
        v_view = V[st][:].rearrange("p (h k) -> p h k", k=VW)
        nc.scalar.dma_start(
            v_view[:, :, DK:VW].unsqueeze(1),
            io["onesw"].ap().unsqueeze(1).unsqueeze(0).partition_broadcast(P),
        )

    # Two PSUM pools, 2 slots x [128, 1024] (2 banks) each = all 8 banks.
    # Pool A: av/bc tiles in the unit loop; pool B: score tiles. Projections
    # draw 2 slots from each.
    poolA = tc.alloc_tile_pool(name="poolA", bufs=2, space="PSUM")
    poolB = tc.alloc_tile_pool(name="poolB", bufs=2, space="PSUM")

    def psum8(pfx):
        pairs = [
            (poolA if s < 2 else poolB).tile(
                [P, 2 * QC], F32, tag="u", name=f"{pfx}_{s}"
            )
            for s in range(NT // 2)
        ]
        return [pairs[t // 2][:, (t % 2) * QC : (t % 2 + 1) * QC] for t in range(NT)]

    # --- phase A: QT / KT projections (transposed, bias per-partition) ---
    for dst, wname, xname, bias in () if "A" not in phases else (
        (QT, "wqT", "xqT", bq_sb),
        (KT, "wkT", "xkT", bk_sb),
    ):
        w_ap = io[wname].ap()
        x_ap = io[xname].ap()
        wts = []  # w blocks stay resident across both c-chunks (loaded once)
        xt0 = None
        for c in range(2):
            ps = psum8(f"ps_{wname}{c}")
            for d in range(NT):
                if "a" in phases and xt0 is not None:
                    xt = xt0
                    wt = wts[0]
                else:
                    xt = stream.tile(
                        [P, QC], BF16, tag="xc", bufs=3, name=f"x_{xname}{c}_{d}"
                    )
                    nc.sync.dma_start(
                        xt[:], x_ap[d * P : (d + 1) * P, c * QC : (c + 1) * QC]
                    )
                    if xt0 is None:
                        xt0 = xt
                if "a" not in phases:
                    if c == 0:
                        wt = stream.tile(
                            [P, D], BF16, tag="big", bufs=8, name=f"w_{wname}_{d}"
                        )
                        nc.sync.dma_start(wt[:], w_ap[d * P : (d + 1) * P, :])
                        wts.append(wt)
                    else:
                        wt = wts[d]
                elif not wts:
                    wt = stream.tile(
                        [P, D], BF16, tag="big", bufs=8, name=f"w_{wname}_0"
                    )
                    nc.sync.dma_start(wt[:], w_ap[0:P, :])
                    wts.append(wt)
                for t in range(NT):
                    nc.tensor.matmul(
                        ps[t][:],
                        lhsT=wt[:, t * P : (t + 1) * P],
                        rhs=xt[:],
                        start=(d == 0),
                        stop=(d == NT - 1),
                    )
            for t in range(NT):
                nc.vector.tensor_scalar_add(
                    dst[t][:, c * QC : (c + 1) * QC], ps[t][:], bias[:, t : t + 1]
                )

    # prefetch the V-projection operands (used ~25us later; DMA has slack)
    if "C" in phases:
        xv_ap = io["xvT"].ap()
        wv_ap = io["wvT"].ap()
        for d in range(NT):
            nc.sync.dma_start(XV[d][:], xv_ap[d * P : (d + 1) * P, :])
            nc.sync.dma_start(WV[d][:], wv_ap[d * P : (d + 1) * P, :])

    # --- scores + exp for one unit (p, qi) -------------------------------
    dsb = tc.alloc_tile_pool(name="dsb", bufs=1)
    ats = {}

    def emit_scores(p, qi):
        qs = slice(qi * QC, (qi + 1) * QC)
        groups = []
        for g in range(4):  # groups of 2 k-blocks -> [128, 1024] psum
            sce = poolB.tile([P, 2 * QC], F32, tag="u", name=f"sce{p}_{qi}_{g}")
            sco = poolB.tile([P, 2 * QC], F32, tag="u", name=f"sco{p}_{qi}_{g}")
            for j in range(2):
                kb = 2 * g + j
                ksl = slice(kb * P, (kb + 1) * P)
                nc.tensor.matmul(
                    sce[:, j * QC : (j + 1) * QC],
                    lhsT=KT[p][0:64, ksl],
                    rhs=QT[p][0:64, qs],
                    start=True,
                    stop=True,
                )
                nc.tensor.matmul(
                    sco[:, j * QC : (j + 1) * QC],
                    lhsT=KT[p][64:128, ksl],
                    rhs=QT[p][64:128, qs],
                    start=True,
                    stop=True,
                    tile_position=(64, 0),
                )
            at_dt = F32R if "F" in phases else BF16
            at_bufs = 14 if "F" in phases else 30
            ae = dsb.tile([P, 2 * QC], at_dt, tag="at", bufs=at_bufs,
                          name=f"ae{p}_{qi}_{g}")
            nc.scalar.activation(ae[:], sce[:], EXP)
            ao = dsb.tile([P, 2 * QC], at_dt, tag="at", bufs=at_bufs,
                          name=f"ao{p}_{qi}_{g}")
            nc.scalar.activation(ao[:], sco[:], EXP)
            groups.append((ae, ao))
        ats[(p, qi)] = groups

    units = [(p, qi) for p in range(NT) for qi in range(2)]
    if "S" in phases and "V" not in phases:
        for p, qi in units:
            emit_scores(p, qi)
    if "S" in phases and "V" in phases:
        for p, qi in units[:LAG]:
            emit_scores(p, qi)

    # --- phase C: V projection by s-strip (1 pool-A slot per strip) ------
    # scores for unit LAG are emitted mid-C so the scalar engine stays fed
    # with exp work through the end of C (its 3 early units only cover ~25us
    # of the ~27us projection).
    for st in range(NT) if "C" in phases else ():
        if st == 4 and "S" in phases and "V" in phases:
            emit_scores(*units[LAG])
        vp = poolA.tile([P, 2 * QC], F32, tag="u", name=f"vps{st}")
        for d in range(NT):
            for c in range(2):
                nc.tensor.matmul(
                    vp[:, c * QC : (c + 1) * QC],
                    lhsT=XV[d][:, st * P : (st + 1) * P],
                    rhs=WV[d][:, c * QC : (c + 1) * QC],
                    start=(d == 0),
                    stop=(d == NT - 1),
                )
        v_out = V[st][:].rearrange("p (h k) -> p h k", k=VW)[:, :, 0:DK]
        ps_v = vp[:].rearrange("p (h k) -> p h k", k=DK)
        bv_v = bv_bc[:].rearrange("p (h k) -> p h k", k=DK)
        nc.vector.tensor_add(v_out, ps_v, bv_v)

    # --- unit loop: attn@V, normalize; scores for unit i+LAG in between --
    for i, (p, qi) in enumerate(units) if "V" in phases else ():
        he, ho = 2 * p, 2 * p + 1
        qs = slice(qi * QC, (qi + 1) * QC)
        groups = ats.pop((p, qi))

        av = poolA.tile([P, 2 * QC], F32, tag="u", name=f"av{p}_{qi}")
        ave = av[:, 0:QC]
        avo = av[:, QC : 2 * QC]
        for g in range(4):
            ae, ao = groups[g]
            for j in range(2):
                kb = 2 * g + j
                nc.tensor.matmul(
                    ave[0:VW, :],
                    lhsT=V[kb][:, he * VW : (he + 1) * VW],
                    rhs=ae[:, j * QC : (j + 1) * QC],
                    start=(kb == 0),
                    stop=(kb == NT - 1),
                )
                nc.tensor.matmul(
                    avo[0:VW, :],
                    lhsT=V[kb][:, ho * VW : (ho + 1) * VW],
                    rhs=ao[:, j * QC : (j + 1) * QC],
                    start=(kb == 0),
                    stop=(kb == NT - 1),
                )

        # one wide reciprocal of the sumexp row (partition 64, both parities)
        rec = dsb.tile([P, 2 * QC], F32, tag="rec", bufs=2, name=f"rec{p}_{qi}")
        nc.vector.reciprocal(rec[64:65, :], av[64:65, :])

        # keep PE busy with the next unit's scores while DVE runs reciprocal
        # (units 0..LAG were emitted before/during C, so the loop emits i+LAG+1)
        if i + LAG + 1 < len(units):
            emit_scores(*units[i + LAG + 1])

        # broadcast the reciprocal row down to 64 partitions via a DRAM
        # bounce (SBUF->DRAM->SBUF partition-broadcast load, the proven
        # bias-load pattern; SP DGE is idle here and this keeps PE/PSUM out
        # of the normalize chain)
        scr = io["recscr"].ap()[i % 2]
        nc.sync.dma_start(scr, rec[64:65, :])
        rbc = dsb.tile([DK, 2 * QC], F32, tag="rbc", bufs=2, name=f"rbc{p}_{qi}")
        nc.sync.dma_start(
            rbc[:].unsqueeze(1), scr.unsqueeze(0).partition_broadcast(DK)
        )
        nc.vector.tensor_mul(OT[p][0:64, qs], ave[0:64, :], rbc[:, 0:QC])
        tmpo = dsb.tile([64, QC], BF16, tag="tmp", bufs=2, name=f"tmpo{p}_{qi}")
        nc.vector.tensor_mul(tmpo[:], avo[0:64, :], rbc[:, QC : 2 * QC])
        nc.sync.dma_start(OT[p][64:128, qs], tmpo[:])

    # --- phase E: output projection out[s, f] = OT.T @ woT + bo ----------
    out_ap = io["out"].ap()
    wo_ap = io["woT"].ap()
    if "E" not in phases:
        # bench-only: drain something comparable to E's output traffic
        srcs = OT if "V" in phases else QT
        for t in range(NT):
            nc.sync.dma_start(out_ap[t * P : (t + 1) * P, :], srcs[t][:, 0:S])
    for c in range(2) if "E" in phases else ():
        fs = slice(c * QC, (c + 1) * QC)
        ps = psum8(f"ps_o{c}")
        for e in range(NT):
            wt = stream.tile([P, QC], BF16, tag="xc", bufs=3, name=f"w_o{c}_{e}")
            nc.sync.dma_start(wt[:], wo_ap[e * P : (e + 1) * P, fs])
            for st in range(NT):
                nc.tensor.matmul(
                    ps[st][:],
                    lhsT=OT[e][:, st * P : (st + 1) * P],
                    rhs=wt[:],
                    start=(e == 0),
                    stop=(e == NT - 1),
                )
        for st in range(NT):
            ob = stream.tile([P, QC], BF16, tag="ob", bufs=2, name=f"ob{c}_{st}")
            nc.vector.tensor_add(ob[:], ps[st][:], bo_bc[:, fs])
            nc.sync.dma_start(out_ap[st * P : (st + 1) * P, fs], ob[:])

    dsb.release()
    poolB.release()
    poolA.release()
    stream.release()
    persist.release()
    const.release()


def build_nc(repeats=1, phases="ASCVE"):
    nc = bacc.Bacc(
        "TRN2",
        target_bir_lowering=False,
        debug=False,
        enable_asserts=False,
        num_devices=NB,
    )
    io = {}
    for name in ("xqT", "xkT", "xvT"):
        io[name] = nc.dram_tensor(name, [D, S], BF16, kind="ExternalInput")
    for name in ("wqT", "wkT", "wvT", "woT"):
        io[name] = nc.dram_tensor(name, [D, D], BF16, kind="ExternalInput")
    for name in ("bqs", "bk", "bv", "bo"):
        io[name] = nc.dram_tensor(name, [D], F32, kind="ExternalInput")
    io["onesw"] = nc.dram_tensor("onesw", [H], BF16, kind="ExternalInput")
    io["out"] = nc.dram_tensor("out", [S, D], BF16, kind="ExternalOutput")
    io["recscr"] = nc.dram_tensor("recscr", [2, 2 * QC], F32, kind="Internal")

    with tile.TileContext(nc) as tc:
        for _ in range(repeats):
            _emit(tc, io, phases)
    nc.compile()
    return nc


_CACHE = {}


def get_nc():
    if "nc" not in _CACHE:
        _CACHE["nc"] = build_nc()
    return _CACHE["nc"]


def make_in_maps(query, key, value, wq, bq, wk, bk, wv, bv, wo, bo):
    f = np.float32
    # fold the 1/sqrt(DK) score scaling into the Q projection (exact: 1/8)
    wqT = (np.asarray(wq, f).T * f(0.125)).astype(NPBF16)
    bqs = np.asarray(bq, f) * f(0.125)
    wkT = np.asarray(wk, f).T.astype(NPBF16)
    wvT = np.asarray(wv, f).T.astype(NPBF16)
    woT = np.asarray(wo, f).T.astype(NPBF16)
    common = {
        "wqT": np.ascontiguousarray(wqT),
        "wkT": np.ascontiguousarray(wkT),
        "wvT": np.ascontiguousarray(wvT),
        "woT": np.ascontiguousarray(woT),
        "bqs": np.ascontiguousarray(bqs),
        "bk": np.ascontiguousarray(np.asarray(bk, f)),
        "bv": np.ascontiguousarray(np.asarray(bv, f)),
        "bo": np.ascontiguousarray(np.asarray(bo, f)),
        "onesw": np.ones(H, NPBF16),
    }
    q = np.asarray(query, f)
    k = np.asarray(key, f)
    v = np.asarray(value, f)
    in_maps = []
    for b in range(NB):
        in_maps.append(
            {
                "xqT": np.ascontiguousarray(q[b].T.astype(NPBF16)),
                "xkT": np.ascontiguousarray(k[b].T.astype(NPBF16)),
                "xvT": np.ascontiguousarray(v[b].T.astype(NPBF16)),
                **common,
            }
        )
    return in_maps


def kernel(
    query,
    key,
    value,
    inputs_attn_mask=None,  # all-ones per spec; masking is a no-op
    wq=None, bq=None, wk=None, bk=None, wv=None, bv=None, wo=None, bo=None,
    **_extra,
):
    nc = get_nc()
    in_maps = make_in_maps(query, key, value, wq, bq, wk, bk, wv, bv, wo, bo)
    res = run_bass_kernel_spmd(nc, in_maps, core_ids=list(range(NB)))
    out = np.stack(
        [np.asarray(res.results[b]["out"]).astype(np.float32) for b in range(NB)],
        axis=0,
    )
    return out


# revision 34
# speedup vs baseline: 1.1147x; 1.1016x over previous
"""Multi-head self-attention (B=8, S=1024, D=1024, H=16) on 8 trn2 cores.

Sharding: pure data-parallel over batch (B=8 -> 1 batch per core, no
collectives). Per-core kernel computes the full attention for one batch.

v2: all matmul operands in bf16 (host-converted; fp32 PSUM accumulation,
rel err ~2.5e-3 vs the 2e-2 gate) and a software-pipelined emission order
that hides the softmax exp (ScalarE, ~131us total, no 16-bit speedup on
ACT) under the tensor-engine work:

  A: QT/KT projections (transposed layout, head dim on partitions),
     by-c psum8 structure, bf16 in/out.
  S: scores+exp for the first LAG units emitted right after A so the
     scalar engine starts while V is still projecting.
  C: V projection by-s-strip (1 psum slot per strip) from SBUF-resident
     xvT/wvT so it coexists with the early score units; V stored with a
     65th ones column per head (attn@V also yields the softmax sum).
  unit loop (16 units = 8 head-pairs x 2 q-chunks): attn@V accumulation,
     one wide reciprocal of the sumexp row (DVE), scores+exp for unit
     i+LAG+1 (keeps PE busy while the reciprocal runs), reciprocal row
     broadcast to 64 partitions via a DRAM bounce (the SWDGE
     partition_broadcast instruction is wrong on HW despite passing
     CoreSim), normalize, odd head DMA-shifted to partitions 64..127
     of OT.
  E: output projection from bf16 OT, bf16 out (host converts to fp32).

PSUM is split into two 2-slot pools (av in A, score tiles in B) so the
strict round-robin slot rotation never makes a PE instruction wait on a
slot whose free depends on a later PE instruction (FIFO inversion
deadlock).

HW notes (differential phase timing; no NTFF profiling in this env):
exp on ACT costs ~1.9us per [128,1024] bf16 tile on HW (1.04 modeled;
fp32 out is ~20% faster but halves the SBUF exp window and slows the
f32r attn@V rhs, net loss). Phase A is not DMA-bound (no-DMA variant
times the same). The `phases` arg builds partial kernels for
differential HW timing ("a" = A without DMA, "F" = fp32 exp tiles).
"""

import sys

for _p in ("/opt/trn_rl_repo", "/root/.axon_site/_ro/trn_rl_repo"):
    if _p not in sys.path:
        sys.path.append(_p)

import numpy as np
import ml_dtypes

import concourse.bass as bass
import concourse.mybir as mybir
import concourse.tile as tile
from concourse import bacc
from concourse.bass_utils import run_bass_kernel_spmd

F32 = mybir.dt.float32
F32R = mybir.dt.float32r
BF16 = mybir.dt.bfloat16
NPBF16 = ml_dtypes.bfloat16
EXP = mybir.ActivationFunctionType.Exp

S = 1024   # sequence length
D = 1024   # model dim
H = 16     # heads
DK = 64    # head dim
P = 128    # partitions
QC = 512   # q-chunk (psum bank free size in fp32)
NT = D // P   # 8 e-tiles / d-tiles / s-tiles
NB = 8     # batches == cores

VW = DK + 1  # 65: V columns per head incl. ones column
LAG = 3      # score/exp units emitted ahead of their attn@V consumer


def _emit(tc, io, phases="ASCVE"):
    nc = tc.nc

    const = tc.alloc_tile_pool(name="const", bufs=1)
    persist = tc.alloc_tile_pool(name="persist", bufs=1)
    stream = tc.alloc_tile_pool(name="stream", bufs=1)

    # --- constants -------------------------------------------------------
    # All const DMAs go through the Activation engine's DGE so the SP DGE can
    # start streaming phase-A x/w tiles immediately (HWDGE descriptor
    # processing is ~625ns each and serializes per engine queue).
    # per-partition bias columns: column t holds bias[t*128:(t+1)*128]
    bq_sb = const.tile([P, NT], F32, tag="bq", name="bq_sb")
    nc.scalar.dma_start(bq_sb[:], io["bqs"].ap().rearrange("(t p) -> p t", p=P))
    bk_sb = const.tile([P, NT], F32, tag="bk", name="bk_sb")
    nc.scalar.dma_start(bk_sb[:], io["bk"].ap().rearrange("(t p) -> p t", p=P))
    # row-broadcast bias tiles [128, D] for biases added along the free dim
    bv_bc = const.tile([P, D], F32, tag="bv", name="bv_bc")
    nc.scalar.dma_start(
        bv_bc[:].unsqueeze(1), io["bv"].ap().unsqueeze(0).partition_broadcast(P)
    )
    bo_bc = const.tile([P, D], F32, tag="bo", name="bo_bc")
    nc.scalar.dma_start(
        bo_bc[:].unsqueeze(1), io["bo"].ap().unsqueeze(0).partition_broadcast(P)
    )


    # --- persistent SBUF tensors (all bf16) ------------------------------
    QT = [persist.tile([P, S], BF16, tag=f"qt{t}", name=f"qt{t}") for t in range(NT)]
    KT = [persist.tile([P, S], BF16, tag=f"kt{t}", name=f"kt{t}") for t in range(NT)]
    # V with a ones column appended per head: [s, 16*65]
    V = [persist.tile([P, H * VW], BF16, tag=f"v{t}", name=f"v{t}") for t in range(NT)]
    OT = [persist.tile([P, S], BF16, tag=f"ot{t}", name=f"ot{t}") for t in range(NT)]
    # SBUF-resident xvT / wvT blocks for the by-strip V projection
    XV = [persist.tile([P, S], BF16, tag=f"xv{t}", name=f"xv{t}") for t in range(NT)]
    WV = [persist.tile([P, D], BF16, tag=f"wv{t}", name=f"wv{t}") for t in range(NT)]

    # ones columns of V (column 64 of each head's 65-wide group)
    for st in range(NT):
        v_view = V[st][:].rearrange("p (h k) -> p h k", k=VW)
        nc.scalar.dma_start(
            v_view[:, :, DK:VW].unsqueeze(1),
            io["onesw"].ap().unsqueeze(1).unsqueeze(0).partition_broadcast(P),
        )

    # Two PSUM pools, 2 slots x [128, 1024] (2 banks) each = all 8 banks.
    # Pool A: av/bc tiles in the unit loop; pool B: score tiles. Projections
    # draw 2 slots from each.
    poolA = tc.alloc_tile_pool(name="poolA", bufs=2, space="PSUM")
    poolB = tc.alloc_tile_pool(name="poolB", bufs=2, space="PSUM")

    def psum8(pfx):
        pairs = [
            (poolA if s < 2 else poolB).tile(
                [P, 2 * QC], F32, tag="u", name=f"{pfx}_{s}"
            )
            for s in range(NT // 2)
        ]
        return [pairs[t // 2][:, (t % 2) * QC : (t % 2 + 1) * QC] for t in range(NT)]

    # --- phase A: QT / KT projections (transposed, bias per-partition) ---
    for dst, wname, xname, bias in () if "A" not in phases else (
        (QT, "wqT", "xqT", bq_sb),
        (KT, "wkT", "xkT", bk_sb),
    ):
        w_ap = io[wname].ap()
        x_ap = io[xname].ap()
        wts = []  # w blocks stay resident across both c-chunks (loaded once)
        xt0 = None
        for c in range(2):
            ps = psum8(f"ps_{wname}{c}")
            for d in range(NT):
                if "a" in phases and xt0 is not None:
                    xt = xt0
                    wt = wts[0]
                else:
                    xt = stream.tile(
                        [P, QC], BF16, tag="xc", bufs=3, name=f"x_{xname}{c}_{d}"
                    )
                    nc.sync.dma_start(
                        xt[:], x_ap[d * P : (d + 1) * P, c * QC : (c + 1) * QC]
                    )
                    if xt0 is None:
                        xt0 = xt
                if "a" not in phases:
                    if c == 0:
                        wt = stream.tile(
                            [P, D], BF16, tag="big", bufs=8, name=f"w_{wname}_{d}"
                        )
                        nc.sync.dma_start(wt[:], w_ap[d * P : (d + 1) * P, :])
                        wts.append(wt)
                    else:
                        wt = wts[d]
                elif not wts:
                    wt = stream.tile(
                        [P, D], BF16, tag="big", bufs=8, name=f"w_{wname}_0"
                    )
                    nc.sync.dma_start(wt[:], w_ap[0:P, :])
                    wts.append(wt)
                for t in range(NT):
                    nc.tensor.matmul(
                        ps[t][:],
                        lhsT=wt[:, t * P : (t + 1) * P],
                        rhs=xt[:],
                        start=(d == 0),
                        stop=(d == NT - 1),
                    )
            for t in range(NT):
                nc.vector.tensor_scalar_add(
                    dst[t][:, c * QC : (c + 1) * QC], ps[t][:], bias[:, t : t + 1]
                )

    # prefetch the V-projection operands (used ~25us later; DMA has slack)
    if "C" in phases:
        xv_ap = io["xvT"].ap()
        wv_ap = io["wvT"].ap()
        for d in range(NT):
            nc.sync.dma_start(XV[d][:], xv_ap[d * P : (d + 1) * P, :])
            nc.sync.dma_start(WV[d][:], wv_ap[d * P : (d + 1) * P, :])

    # --- scores + exp for one unit (p, qi) -------------------------------
    dsb = tc.alloc_tile_pool(name="dsb", bufs=1)
    ats = {}

    def emit_scores(p, qi):
        qs = slice(qi * QC, (qi + 1) * QC)
        groups = []
        for g in range(4):  # groups of 2 k-blocks -> [128, 1024] psum
            sce = poolB.tile([P, 2 * QC], F32, tag="u", name=f"sce{p}_{qi}_{g}")
            sco = poolB.tile([P, 2 * QC], F32, tag="u", name=f"sco{p}_{qi}_{g}")
            for j in range(2):
                kb = 2 * g + j
                ksl = slice(kb * P, (kb + 1) * P)
                nc.tensor.matmul(
                    sce[:, j * QC : (j + 1) * QC],
                    lhsT=KT[p][0:64, ksl],
                    rhs=QT[p][0:64, qs],
                    start=True,
                    stop=True,
                )
                nc.tensor.matmul(
                    sco[:, j * QC : (j + 1) * QC],
                    lhsT=KT[p][64:128, ksl],
                    rhs=QT[p][64:128, qs],
                    start=True,
                    stop=True,
                    tile_position=(64, 0),
                )
            at_dt = F32R if "F" in phases else BF16
            at_bufs = 14 if "F" in phases else 30
            ae = dsb.tile([P, 2 * QC], at_dt, tag="at", bufs=at_bufs,
                          name=f"ae{p}_{qi}_{g}")
            nc.scalar.activation(ae[:], sce[:], EXP)
            ao = dsb.tile([P, 2 * QC], at_dt, tag="at", bufs=at_bufs,
                          name=f"ao{p}_{qi}_{g}")
            nc.scalar.activation(ao[:], sco[:], EXP)
            groups.append((ae, ao))
        ats[(p, qi)] = groups

    units = [(p, qi) for p in range(NT) for qi in range(2)]
    if "S" in phases and "V" not in phases:
        for p, qi in units:
            emit_scores(p, qi)
    if "S" in phases and "V" in phases:
        for p, qi in units[:LAG]:
            emit_scores(p, qi)

    # --- phase C: V projection by s-strip (1 pool-A slot per strip) ------
    # scores for unit LAG are emitted mid-C so the scalar engine stays fed
    # with exp work through the end of C (its 3 early units only cover ~25us
    # of the ~27us projection).
    for st in range(NT) if "C" in phases else ():
        if st == 4 and "S" in phases and "V" in phases:
            emit_scores(*units[LAG])
        vp = poolA.tile([P, 2 * QC], F32, tag="u", name=f"vps{st}")
        for d in range(NT):
            for c in range(2):
                nc.tensor.matmul(
                    vp[:, c * QC : (c + 1) * QC],
                    lhsT=XV[d][:, st * P : (st + 1) * P],
                    rhs=WV[d][:, c * QC : (c + 1) * QC],
                    start=(d == 0),
                    stop=(d == NT - 1),
                )
        v_out = V[st][:].rearrange("p (h k) -> p h k", k=VW)[:, :, 0:DK]
        ps_v = vp[:].rearrange("p (h k) -> p h k", k=DK)
        bv_v = bv_bc[:].rearrange("p (h k) -> p h k", k=DK)
        nc.vector.tensor_add(v_out, ps_v, bv_v)

    # --- unit loop: attn@V, normalize; scores for unit i+LAG in between --
    for i, (p, qi) in enumerate(units) if "V" in phases else ():
        he, ho = 2 * p, 2 * p + 1
        qs = slice(qi * QC, (qi + 1) * QC)
        groups = ats.pop((p, qi))

        av = poolA.tile([P, 2 * QC], F32, tag="u", name=f"av{p}_{qi}")
        ave = av[:, 0:QC]
        avo = av[:, QC : 2 * QC]
        for g in range(4):
            ae, ao = groups[g]
            for j in range(2):
                kb = 2 * g + j
                nc.tensor.matmul(
                    ave[0:VW, :],
                    lhsT=V[kb][:, he * VW : (he + 1) * VW],
                    rhs=ae[:, j * QC : (j + 1) * QC],
                    start=(kb == 0),
                    stop=(kb == NT - 1),
                )
                nc.tensor.matmul(
                    avo[0:VW, :],
                    lhsT=V[kb][:, ho * VW : (ho + 1) * VW],
                    rhs=ao[:, j * QC : (j + 1) * QC],
                    start=(kb == 0),
                    stop=(kb == NT - 1),
                )

        # one wide reciprocal of the sumexp row (partition 64, both parities)
        rec = dsb.tile([P, 2 * QC], F32, tag="rec", bufs=2, name=f"rec{p}_{qi}")
        nc.vector.reciprocal(rec[64:65, :], av[64:65, :])

        # keep PE busy with the next unit's scores while DVE runs reciprocal
        # (units 0..LAG were emitted before/during C, so the loop emits i+LAG+1)
        if i + LAG + 1 < len(units):
            emit_scores(*units[i + LAG + 1])

        # broadcast the reciprocal row down to 64 partitions via a DRAM
        # bounce (SBUF->DRAM->SBUF partition-broadcast load, the proven
        # bias-load pattern; SP DGE is idle here and this keeps PE/PSUM out
        # of the normalize chain)
        scr = io["recscr"].ap()[i % 2]
        nc.sync.dma_start(scr, rec[64:65, :])
        rbc = dsb.tile([DK, 2 * QC], F32, tag="rbc", bufs=2, name=f"rbc{p}_{qi}")
        nc.sync.dma_start(
            rbc[:].unsqueeze(1), scr.unsqueeze(0).partition_broadcast(DK)
        )
        nc.vector.tensor_mul(OT[p][0:64, qs], ave[0:64, :], rbc[:, 0:QC])
        tmpo = dsb.tile([64, QC], BF16, tag="tmp", bufs=2, name=f"tmpo{p}_{qi}")
        nc.vector.tensor_mul(tmpo[:], avo[0:64, :], rbc[:, QC : 2 * QC])
        nc.sync.dma_start(OT[p][64:128, qs], tmpo[:])

    # --- phase E: output projection out[s, f] = OT.T @ woT + bo ----------
    out_ap = io["out"].ap()
    wo_ap = io["woT"].ap()
    if "E" not in phases:
        # bench-only: drain something comparable to E's output traffic
        srcs = OT if "V" in phases else QT
        for t in range(NT):
            nc.sync.dma_start(out_ap[t * P : (t + 1) * P, :], srcs[t][:, 0:S])
    for c in range(2) if "E" in phases else ():
        fs = slice(c * QC, (c + 1) * QC)
        ps = psum8(f"ps_o{c}")
        for e in range(NT):
            wt = stream.tile([P, QC], BF16, tag="xc", bufs=3, name=f"w_o{c}_{e}")
            nc.sync.dma_start(wt[:], wo_ap[e * P : (e + 1) * P, fs])
            for st in range(NT):
                nc.tensor.matmul(
                    ps[st][:],
                    lhsT=OT[e][:, st * P : (st + 1) * P],
                    rhs=wt[:],
                    start=(e == 0),
                    stop=(e == NT - 1),
                )
        for st in range(NT):
            ob = stream.tile([P, QC], BF16, tag="ob", bufs=2, name=f"ob{c}_{st}")
            nc.vector.tensor_add(ob[:], ps[st][:], bo_bc[:, fs])
            nc.sync.dma_start(out_ap[st * P : (st + 1) * P, fs], ob[:])

    dsb.release()
    poolB.release()
    poolA.release()
    stream.release()
    persist.release()
    const.release()


def build_nc(repeats=1, phases="ASCVE"):
    nc = bacc.Bacc(
        "TRN2",
        target_bir_lowering=False,
        debug=False,
        enable_asserts=False,
        num_devices=NB,
    )
    io = {}
    for name in ("xqT", "xkT", "xvT"):
        io[name] = nc.dram_tensor(name, [D, S], BF16, kind="ExternalInput")
    for name in ("wqT", "wkT", "wvT", "woT"):
        io[name] = nc.dram_tensor(name, [D, D], BF16, kind="ExternalInput")
    for name in ("bqs", "bk", "bv", "bo"):
        io[name] = nc.dram_tensor(name, [D], F32, kind="ExternalInput")
    io["onesw"] = nc.dram_tensor("onesw", [H], BF16, kind="ExternalInput")
    io["out"] = nc.dram_tensor("out", [S, D], BF16, kind="ExternalOutput")
    io["recscr"] = nc.dram_tensor("recscr", [2, 2 * QC], F32, kind="Internal")

    with tile.TileContext(nc) as tc:
        for _ in range(repeats):
            _emit(tc, io, phases)
    nc.compile()
    return nc


_CACHE = {}


def get_nc():
    if "nc" not in _CACHE:
        _CACHE["nc"] = build_nc()
    return _CACHE["nc"]


def make_in_maps(query, key, value, wq, bq, wk, bk, wv, bv, wo, bo):
    f = np.float32
    # fold the 1/sqrt(DK) score scaling into the Q projection (exact: 1/8)
    wqT = (np.asarray(wq, f).T * f(0.125)).astype(NPBF16)
    bqs = np.asarray(bq, f) * f(0.125)
    wkT = np.asarray(wk, f).T.astype(NPBF16)
    wvT = np.asarray(wv, f).T.astype(NPBF16)
    woT = np.asarray(wo, f).T.astype(NPBF16)
    common = {
        "wqT": np.ascontiguousarray(wqT),
        "wkT": np.ascontiguousarray(wkT),
        "wvT": np.ascontiguousarray(wvT),
        "woT": np.ascontiguousarray(woT),
        "bqs": np.ascontiguousarray(bqs),
        "bk": np.ascontiguousarray(np.asarray(bk, f)),
        "bv": np.ascontiguousarray(np.asarray(bv, f)),
        "bo": np.ascontiguousarray(np.asarray(bo, f)),
        "onesw": np.ones(H, NPBF16),
    }
    q = np.asarray(query, f)
    k = np.asarray(key, f)
    v = np.asarray(value, f)
    in_maps = []
    for b in range(NB):
        in_maps.append(
            {
                "xqT": np.ascontiguousarray(q[b].T.astype(NPBF16)),
                "xkT": np.ascontiguousarray(k[b].T.astype(NPBF16)),
                "xvT": np.ascontiguousarray(v[b].T.astype(NPBF16)),
                **common,
            }
        )
    return in_maps


def kernel(
    query,
    key,
    value,
    inputs_attn_mask=None,  # all-ones per spec; masking is a no-op
    wq=None, bq=None, wk=None, bk=None, wv=None, bv=None, wo=None, bo=None,
    **_extra,
):
    nc = get_nc()
    in_maps = make_in_maps(query, key, value, wq, bq, wk, bk, wv, bv, wo, bo)
    res = run_bass_kernel_spmd(nc, in_maps, core_ids=list(range(NB)))
    out = np.stack(
        [np.asarray(res.results[b]["out"]).astype(np.float32) for b in range(NB)],
        axis=0,
    )
    return out


# revision 36
# speedup vs baseline: 1.2012x; 1.0776x over previous
"""Multi-head self-attention (B=8, S=1024, D=1024, H=16) on 8 trn2 cores.

Sharding: pure data-parallel over batch (B=8 -> 1 batch per core, no
collectives). Per-core kernel computes the full attention for one batch.

v2: all matmul operands in bf16 (host-converted; fp32 PSUM accumulation,
rel err ~2.5e-3 vs the 2e-2 gate) and a software-pipelined emission order
that hides the softmax exp (ScalarE, ~131us total, no 16-bit speedup on
ACT) under the tensor-engine work:

  A: QT/KT projections (transposed layout, head dim on partitions),
     by-c psum8 structure, bf16 in/out.
  S: scores+exp for the first LAG units emitted right after A so the
     scalar engine starts while V is still projecting.
  C: V projection by-s-strip (1 psum slot per strip) from SBUF-resident
     xvT/wvT so it coexists with the early score units; V stored with a
     65th ones column per head (attn@V also yields the softmax sum).
  unit loop (16 units = 8 head-pairs x 2 q-chunks): attn@V accumulation,
     one wide reciprocal of the sumexp row (DVE), scores+exp for unit
     i+LAG+1 (keeps PE busy while the reciprocal runs), reciprocal row
     broadcast to 64 partitions via a DRAM bounce (the SWDGE
     partition_broadcast instruction is wrong on HW despite passing
     CoreSim), normalize, odd head DMA-shifted to partitions 64..127
     of OT.
  E: output projection from bf16 OT, bf16 out (host converts to fp32).

PSUM is split into two 2-slot pools (av in A, score tiles in B) so the
strict round-robin slot rotation never makes a PE instruction wait on a
slot whose free depends on a later PE instruction (FIFO inversion
deadlock).

HW notes (differential phase timing; no NTFF profiling in this env):
exp on ACT costs ~1.9us per [128,1024] bf16 tile on HW (1.04 modeled;
fp32 out is ~20% faster but halves the SBUF exp window and slows the
f32r attn@V rhs, net loss). Phase A is not DMA-bound (no-DMA variant
times the same). The `phases` arg builds partial kernels for
differential HW timing ("a" = A without DMA, "F" = fp32 exp tiles).
"""

import sys

for _p in ("/opt/trn_rl_repo", "/root/.axon_site/_ro/trn_rl_repo"):
    if _p not in sys.path:
        sys.path.append(_p)

import numpy as np
import ml_dtypes

import concourse.bass as bass
import concourse.mybir as mybir
import concourse.tile as tile
from concourse import bacc
from concourse.bass_utils import run_bass_kernel_spmd

F32 = mybir.dt.float32
F32R = mybir.dt.float32r
BF16 = mybir.dt.bfloat16
NPBF16 = ml_dtypes.bfloat16
EXP = mybir.ActivationFunctionType.Exp

S = 1024   # sequence length
D = 1024   # model dim
H = 16     # heads
DK = 64    # head dim
P = 128    # partitions
QC = 512   # q-chunk (psum bank free size in fp32)
NT = D // P   # 8 e-tiles / d-tiles / s-tiles
NB = 8     # batches == cores

VW = DK + 1  # 65: V columns per head incl. ones column
LAG = 3      # score/exp units emitted ahead of their attn@V consumer


def _emit(tc, io, phases="ASCVE"):
    nc = tc.nc

    const = tc.alloc_tile_pool(name="const", bufs=1)
    persist = tc.alloc_tile_pool(name="persist", bufs=1)
    stream = tc.alloc_tile_pool(name="stream", bufs=1)

    # --- constants -------------------------------------------------------
    # All const DMAs go through the Activation engine's DGE so the SP DGE can
    # start streaming phase-A x/w tiles immediately (HWDGE descriptor
    # processing is ~625ns each and serializes per engine queue).
    # per-partition bias columns: column t holds bias[t*128:(t+1)*128]
    bq_sb = const.tile([P, NT], F32, tag="bq", name="bq_sb")
    nc.scalar.dma_start(bq_sb[:], io["bqs"].ap().rearrange("(t p) -> p t", p=P))
    bk_sb = const.tile([P, NT], F32, tag="bk", name="bk_sb")
    nc.scalar.dma_start(bk_sb[:], io["bk"].ap().rearrange("(t p) -> p t", p=P))
    # row-broadcast bias tiles [128, D] for biases added along the free dim
    bv_bc = const.tile([P, D], F32, tag="bv", name="bv_bc")
    nc.scalar.dma_start(
        bv_bc[:].unsqueeze(1), io["bv"].ap().unsqueeze(0).partition_broadcast(P)
    )
    bo_bc = const.tile([P, D], F32, tag="bo", name="bo_bc")
    nc.scalar.dma_start(
        bo_bc[:].unsqueeze(1), io["bo"].ap().unsqueeze(0).partition_broadcast(P)
    )


    # --- persistent SBUF tensors (all bf16) ------------------------------
    QT = [persist.tile([P, S], BF16, tag=f"qt{t}", name=f"qt{t}") for t in range(NT)]
    KT = [persist.tile([P, S], BF16, tag=f"kt{t}", name=f"kt{t}") for t in range(NT)]
    # V with a ones column appended per head: [s, 16*65]
    V = [persist.tile([P, H * VW], BF16, tag=f"v{t}", name=f"v{t}") for t in range(NT)]
    OT = [persist.tile([P, S], BF16, tag=f"ot{t}", name=f"ot{t}") for t in range(NT)]
    # SBUF-resident xvT / wvT blocks for the by-strip V projection
    XV = [persist.tile([P, S], BF16, tag=f"xv{t}", name=f"xv{t}") for t in range(NT)]
    WV = [persist.tile([P, D], BF16, tag=f"wv{t}", name=f"wv{t}") for t in range(NT)]

    # ones columns of V (column 64 of each head's 65-wide group)
    for st in range(NT):
        v_view = V[st][:].rearrange("p (h k) -> p h k", k=VW)
        nc.scalar.dma_start(
            v_view[:, :, DK:VW].unsqueeze(1),
            io["onesw"].ap().unsqueeze(1).unsqueeze(0).partition_broadcast(P),
        )

    # Two PSUM pools, 2 slots x [128, 1024] (2 banks) each = all 8 banks.
    # Pool A: av/bc tiles in the unit loop; pool B: score tiles. Projections
    # draw 2 slots from each.
    poolA = tc.alloc_tile_pool(name="poolA", bufs=2, space="PSUM")
    poolB = tc.alloc_tile_pool(name="poolB", bufs=2, space="PSUM")

    def psum8(pfx):
        pairs = [
            (poolA if s < 2 else poolB).tile(
                [P, 2 * QC], F32, tag="u", name=f"{pfx}_{s}"
            )
            for s in range(NT // 2)
        ]
        return [pairs[t // 2][:, (t % 2) * QC : (t % 2 + 1) * QC] for t in range(NT)]

    # --- phase A: QT / KT projections (transposed, bias per-partition) ---
    # The c0 halves of Q and K are computed FIRST: score groups g0/g1 of the
    # qi=0 units only need QT cols 0:512 and KT k-blocks 0..3, so their
    # exps (the scalar engine is the structural bottleneck, ~1.9us/tile on
    # HW) start while the c1 halves are still projecting. c1 runs in two
    # 2-slot passes on pool A only, leaving pool B free for score tiles.
    dsb = tc.alloc_tile_pool(name="dsb", bufs=1)
    ats = {}

    def emit_scores_half(p, qi, half):
        qs = slice(qi * QC, (qi + 1) * QC)
        groups = ats.setdefault((p, qi), [])
        for g in ((0, 1) if half == 0 else (2, 3)):
            sce = poolB.tile([P, 2 * QC], F32, tag="u", name=f"sce{p}_{qi}_{g}")
            sco = poolB.tile([P, 2 * QC], F32, tag="u", name=f"sco{p}_{qi}_{g}")
            for j in range(2):
                kb = 2 * g + j
                ksl = slice(kb * P, (kb + 1) * P)
                nc.tensor.matmul(
                    sce[:, j * QC : (j + 1) * QC],
                    lhsT=KT[p][0:64, ksl],
                    rhs=QT[p][0:64, qs],
                    start=True,
                    stop=True,
                )
                nc.tensor.matmul(
                    sco[:, j * QC : (j + 1) * QC],
                    lhsT=KT[p][64:128, ksl],
                    rhs=QT[p][64:128, qs],
                    start=True,
                    stop=True,
                    tile_position=(64, 0),
                )
            at_dt = F32R if "F" in phases else BF16
            at_bufs = 14 if "F" in phases else 30
            ae = dsb.tile([P, 2 * QC], at_dt, tag="at", bufs=at_bufs,
                          name=f"ae{p}_{qi}_{g}")
            nc.scalar.activation(ae[:], sce[:], EXP)
            ao = dsb.tile([P, 2 * QC], at_dt, tag="at", bufs=at_bufs,
                          name=f"ao{p}_{qi}_{g}")
            nc.scalar.activation(ao[:], sco[:], EXP)
            groups.append((ae, ao))

    def emit_scores(p, qi):
        emit_scores_half(p, qi, 0)
        emit_scores_half(p, qi, 1)

    def a_c0(dst, wname, xname, bias):
        w_ap = io[wname].ap()
        x_ap = io[xname].ap()
        ps = psum8(f"ps_{wname}0")
        for d in range(NT):
            xt = stream.tile([P, QC], BF16, tag="xc", bufs=3,
                             name=f"x_{xname}0_{d}")
            nc.sync.dma_start(xt[:], x_ap[d * P : (d + 1) * P, 0:QC])
            wt = stream.tile([P, D], BF16, tag="big", bufs=8,
                             name=f"w_{wname}0_{d}")
            nc.sync.dma_start(wt[:], w_ap[d * P : (d + 1) * P, :])
            for t in range(NT):
                nc.tensor.matmul(
                    ps[t][:],
                    lhsT=wt[:, t * P : (t + 1) * P],
                    rhs=xt[:],
                    start=(d == 0),
                    stop=(d == NT - 1),
                )
        for t in range(NT):
            nc.vector.tensor_scalar_add(
                dst[t][:, 0:QC], ps[t][:], bias[:, t : t + 1]
            )

    def a_c1(dst, wname, xname, bias):
        w_ap = io[wname].ap()
        x_ap = io[xname].ap()
        for half in range(2):
            tiles = [
                poolA.tile([P, 2 * QC], F32, tag="u", name=f"pc1_{wname}{half}_{s}")
                for s in range(2)
            ]
            ps4 = [tiles[k // 2][:, (k % 2) * QC : (k % 2 + 1) * QC]
                   for k in range(4)]
            for d in range(NT):
                xt = stream.tile([P, QC], BF16, tag="xc", bufs=3,
                                 name=f"x_{xname}1{half}_{d}")
                nc.sync.dma_start(xt[:], x_ap[d * P : (d + 1) * P, QC : 2 * QC])
                wt = stream.tile([P, D], BF16, tag="big", bufs=8,
                                 name=f"w_{wname}1{half}_{d}")
                nc.sync.dma_start(wt[:], w_ap[d * P : (d + 1) * P, :])
                for k in range(4):
                    t = half * 4 + k
                    nc.tensor.matmul(
                        ps4[k],
                        lhsT=wt[:, t * P : (t + 1) * P],
                        rhs=xt[:],
                        start=(d == 0),
                        stop=(d == NT - 1),
                    )
            for k in range(4):
                t = half * 4 + k
                nc.vector.tensor_scalar_add(
                    dst[t][:, QC : 2 * QC], ps4[k][:], bias[:, t : t + 1]
                )

    units = [(p, 0) for p in range(NT)] + [(p, 1) for p in range(NT)]
    pipelined = "S" in phases and "V" in phases and "A" in phases
    N_EARLY = 5   # early qi=0 g0/g1 half-units (4 exp tiles each, cap 30)
    AT_CAP = 28

    if "A" in phases:
        a_c0(QT, "wqT", "xqT", bq_sb)
        a_c0(KT, "wkT", "xkT", bk_sb)
        if pipelined:
            for p in range(N_EARLY):
                emit_scores_half(p, 0, 0)
        a_c1(QT, "wqT", "xqT", bq_sb)
        a_c1(KT, "wkT", "xkT", bk_sb)

    # prefetch the V-projection operands (phase C hides under the exp stretch)
    if "C" in phases:
        xv_ap = io["xvT"].ap()
        wv_ap = io["wvT"].ap()
        for d in range(NT):
            nc.sync.dma_start(XV[d][:], xv_ap[d * P : (d + 1) * P, :])
            nc.sync.dma_start(WV[d][:], wv_ap[d * P : (d + 1) * P, :])

    # chunk scheduler state: pending chunks in strict consumption-need order
    if pipelined:
        pend = [(p, 0, 1) for p in range(N_EARLY)]
        for p in range(N_EARLY, NT):
            pend += [(p, 0, 0), (p, 0, 1)]
        for p in range(NT):
            pend += [(p, 1, 0), (p, 1, 1)]
        state = {"cur": 0, "inflight": 4 * N_EARLY}

        def emit_chunk():
            p, qi, h = pend[state["cur"]]
            emit_scores_half(p, qi, h)
            state["cur"] += 1
            state["inflight"] += 4

        def top_up():
            while state["cur"] < len(pend) and state["inflight"] + 4 <= AT_CAP:
                emit_chunk()

        top_up()  # keep ACT fed through phase C
    elif "S" in phases:
        for p, qi in units:
            emit_scores(p, qi)

    # --- phase C: V projection by s-strip (1 pool-A slot per strip) ------
    # scores for unit LAG are emitted mid-C so the scalar engine stays fed
    # with exp work through the end of C (its 3 early units only cover ~25us
    # of the ~27us projection).
    for st in range(NT) if "C" in phases else ():
        vp = poolA.tile([P, 2 * QC], F32, tag="u", name=f"vps{st}")
        for d in range(NT):
            for c in range(2):
                nc.tensor.matmul(
                    vp[:, c * QC : (c + 1) * QC],
                    lhsT=XV[d][:, st * P : (st + 1) * P],
                    rhs=WV[d][:, c * QC : (c + 1) * QC],
                    start=(d == 0),
                    stop=(d == NT - 1),
                )
        v_out = V[st][:].rearrange("p (h k) -> p h k", k=VW)[:, :, 0:DK]
        ps_v = vp[:].rearrange("p (h k) -> p h k", k=DK)
        bv_v = bv_bc[:].rearrange("p (h k) -> p h k", k=DK)
        nc.vector.tensor_add(v_out, ps_v, bv_v)

    # --- unit loop: attn@V, normalize; scores for unit i+LAG in between --
    for i, (p, qi) in enumerate(units) if "V" in phases else ():
        he, ho = 2 * p, 2 * p + 1
        qs = slice(qi * QC, (qi + 1) * QC)
        while len(ats.get((p, qi), [])) < 4:  # force-complete this unit
            emit_chunk()
        groups = ats.pop((p, qi))

        av = poolA.tile([P, 2 * QC], F32, tag="u", name=f"av{p}_{qi}")
        ave = av[:, 0:QC]
        avo = av[:, QC : 2 * QC]
        for g in range(4):
            ae, ao = groups[g]
            for j in range(2):
                kb = 2 * g + j
                nc.tensor.matmul(
                    ave[0:VW, :],
                    lhsT=V[kb][:, he * VW : (he + 1) * VW],
                    rhs=ae[:, j * QC : (j + 1) * QC],
                    start=(kb == 0),
                    stop=(kb == NT - 1),
                )
                nc.tensor.matmul(
                    avo[0:VW, :],
                    lhsT=V[kb][:, ho * VW : (ho + 1) * VW],
                    rhs=ao[:, j * QC : (j + 1) * QC],
                    start=(kb == 0),
                    stop=(kb == NT - 1),
                )

        # one wide reciprocal of the sumexp row (partition 64, both parities)
        rec = dsb.tile([P, 2 * QC], F32, tag="rec", bufs=2, name=f"rec{p}_{qi}")
        nc.vector.reciprocal(rec[64:65, :], av[64:65, :])

        # keep PE busy with upcoming units' scores while DVE runs reciprocal
        state["inflight"] -= 8
        top_up()

        # broadcast the reciprocal row down to 64 partitions via a DRAM
        # bounce (SBUF->DRAM->SBUF partition-broadcast load, the proven
        # bias-load pattern; SP DGE is idle here and this keeps PE/PSUM out
        # of the normalize chain)
        scr = io["recscr"].ap()[i % 2]
        nc.sync.dma_start(scr, rec[64:65, :])
        rbc = dsb.tile([DK, 2 * QC], F32, tag="rbc", bufs=2, name=f"rbc{p}_{qi}")
        nc.sync.dma_start(
            rbc[:].unsqueeze(1), scr.unsqueeze(0).partition_broadcast(DK)
        )
        nc.vector.tensor_mul(OT[p][0:64, qs], ave[0:64, :], rbc[:, 0:QC])
        tmpo = dsb.tile([64, QC], BF16, tag="tmp", bufs=2, name=f"tmpo{p}_{qi}")
        nc.vector.tensor_mul(tmpo[:], avo[0:64, :], rbc[:, QC : 2 * QC])
        nc.sync.dma_start(OT[p][64:128, qs], tmpo[:])

    # --- phase E: output projection out[s, f] = OT.T @ woT + bo ----------
    out_ap = io["out"].ap()
    wo_ap = io["woT"].ap()
    if "E" not in phases:
        # bench-only: drain something comparable to E's output traffic
        srcs = OT if "V" in phases else QT
        for t in range(NT):
            nc.sync.dma_start(out_ap[t * P : (t + 1) * P, :], srcs[t][:, 0:S])
    for c in range(2) if "E" in phases else ():
        fs = slice(c * QC, (c + 1) * QC)
        ps = psum8(f"ps_o{c}")
        for e in range(NT):
            wt = stream.tile([P, QC], BF16, tag="xc", bufs=3, name=f"w_o{c}_{e}")
            nc.sync.dma_start(wt[:], wo_ap[e * P : (e + 1) * P, fs])
            for st in range(NT):
                nc.tensor.matmul(
                    ps[st][:],
                    lhsT=OT[e][:, st * P : (st + 1) * P],
                    rhs=wt[:],
                    start=(e == 0),
                    stop=(e == NT - 1),
                )
        for st in range(NT):
            ob = stream.tile([P, QC], BF16, tag="ob", bufs=2, name=f"ob{c}_{st}")
            nc.vector.tensor_add(ob[:], ps[st][:], bo_bc[:, fs])
            nc.sync.dma_start(out_ap[st * P : (st + 1) * P, fs], ob[:])

    dsb.release()
    poolB.release()
    poolA.release()
    stream.release()
    persist.release()
    const.release()


def build_nc(repeats=1, phases="ASCVE"):
    nc = bacc.Bacc(
        "TRN2",
        target_bir_lowering=False,
        debug=False,
        enable_asserts=False,
        num_devices=NB,
    )
    io = {}
    for name in ("xqT", "xkT", "xvT"):
        io[name] = nc.dram_tensor(name, [D, S], BF16, kind="ExternalInput")
    for name in ("wqT", "wkT", "wvT", "woT"):
        io[name] = nc.dram_tensor(name, [D, D], BF16, kind="ExternalInput")
    for name in ("bqs", "bk", "bv", "bo"):
        io[name] = nc.dram_tensor(name, [D], F32, kind="ExternalInput")
    io["onesw"] = nc.dram_tensor("onesw", [H], BF16, kind="ExternalInput")
    io["out"] = nc.dram_tensor("out", [S, D], BF16, kind="ExternalOutput")
    io["recscr"] = nc.dram_tensor("recscr", [2, 2 * QC], F32, kind="Internal")

    with tile.TileContext(nc) as tc:
        for _ in range(repeats):
            _emit(tc, io, phases)
    nc.compile()
    return nc


_CACHE = {}


def get_nc():
    if "nc" not in _CACHE:
        _CACHE["nc"] = build_nc()
    return _CACHE["nc"]


def make_in_maps(query, key, value, wq, bq, wk, bk, wv, bv, wo, bo):
    f = np.float32
    # fold the 1/sqrt(DK) score scaling into the Q projection (exact: 1/8)
    wqT = (np.asarray(wq, f).T * f(0.125)).astype(NPBF16)
    bqs = np.asarray(bq, f) * f(0.125)
    wkT = np.asarray(wk, f).T.astype(NPBF16)
    wvT = np.asarray(wv, f).T.astype(NPBF16)
    woT = np.asarray(wo, f).T.astype(NPBF16)
    common = {
        "wqT": np.ascontiguousarray(wqT),
        "wkT": np.ascontiguousarray(wkT),
        "wvT": np.ascontiguousarray(wvT),
        "woT": np.ascontiguousarray(woT),
        "bqs": np.ascontiguousarray(bqs),
        "bk": np.ascontiguousarray(np.asarray(bk, f)),
        "bv": np.ascontiguousarray(np.asarray(bv, f)),
        "bo": np.ascontiguousarray(np.asarray(bo, f)),
        "onesw": np.ones(H, NPBF16),
    }
    q = np.asarray(query, f)
    k = np.asarray(key, f)
    v = np.asarray(value, f)
    in_maps = []
    for b in range(NB):
        in_maps.append(
            {
                "xqT": np.ascontiguousarray(q[b].T.astype(NPBF16)),
                "xkT": np.ascontiguousarray(k[b].T.astype(NPBF16)),
                "xvT": np.ascontiguousarray(v[b].T.astype(NPBF16)),
                **common,
            }
        )
    return in_maps


def kernel(
    query,
    key,
    value,
    inputs_attn_mask=None,  # all-ones per spec; masking is a no-op
    wq=None, bq=None, wk=None, bk=None, wv=None, bv=None, wo=None, bo=None,
    **_extra,
):
    nc = get_nc()
    in_maps = make_in_maps(query, key, value, wq, bq, wk, bk, wv, bv, wo, bo)
    res = run_bass_kernel_spmd(nc, in_maps, core_ids=list(range(NB)))
    out = np.stack(
        [np.asarray(res.results[b]["out"]).astype(np.float32) for b in range(NB)],
        axis=0,
    )
    return out


# revision 38
# speedup vs baseline: 1.2197x; 1.0154x over previous
"""Multi-head self-attention (B=8, S=1024, D=1024, H=16) on 8 trn2 cores.

Sharding: pure data-parallel over batch (B=8 -> 1 batch per core, no
collectives). Per-core kernel computes the full attention for one batch.

v2: all matmul operands in bf16 (host-converted; fp32 PSUM accumulation,
rel err ~2.5e-3 vs the 2e-2 gate) and a software-pipelined emission order
that hides the softmax exp (ScalarE, ~131us total, no 16-bit speedup on
ACT) under the tensor-engine work:

  A: QT/KT projections (transposed layout, head dim on partitions),
     by-c psum8 structure, bf16 in/out.
  S: scores+exp for the first LAG units emitted right after A so the
     scalar engine starts while V is still projecting.
  C: V projection by-s-strip (1 psum slot per strip) from SBUF-resident
     xvT/wvT so it coexists with the early score units; V stored with a
     65th ones column per head (attn@V also yields the softmax sum).
  unit loop (16 units = 8 head-pairs x 2 q-chunks): attn@V accumulation,
     one wide reciprocal of the sumexp row (DVE), scores+exp for unit
     i+LAG+1 (keeps PE busy while the reciprocal runs), reciprocal row
     broadcast to 64 partitions via a DRAM bounce (the SWDGE
     partition_broadcast instruction is wrong on HW despite passing
     CoreSim), normalize, odd head DMA-shifted to partitions 64..127
     of OT.
  E: output projection from bf16 OT, bf16 out (host converts to fp32).

PSUM is split into two 2-slot pools (av in A, score tiles in B) so the
strict round-robin slot rotation never makes a PE instruction wait on a
slot whose free depends on a later PE instruction (FIFO inversion
deadlock).

HW notes (differential phase timing; no NTFF profiling in this env):
exp on ACT costs ~1.9us per [128,1024] bf16 tile on HW (1.04 modeled;
fp32 out is ~20% faster but halves the SBUF exp window and slows the
f32r attn@V rhs, net loss). Phase A is not DMA-bound (no-DMA variant
times the same). The `phases` arg builds partial kernels for
differential HW timing ("a" = A without DMA, "F" = fp32 exp tiles).
"""

import sys

for _p in ("/opt/trn_rl_repo", "/root/.axon_site/_ro/trn_rl_repo"):
    if _p not in sys.path:
        sys.path.append(_p)

import numpy as np
import ml_dtypes

import concourse.bass as bass
import concourse.mybir as mybir
import concourse.tile as tile
from concourse import bacc
from concourse.bass_utils import run_bass_kernel_spmd

F32 = mybir.dt.float32
F32R = mybir.dt.float32r
BF16 = mybir.dt.bfloat16
NPBF16 = ml_dtypes.bfloat16
EXP = mybir.ActivationFunctionType.Exp

S = 1024   # sequence length
D = 1024   # model dim
H = 16     # heads
DK = 64    # head dim
P = 128    # partitions
QC = 512   # q-chunk (psum bank free size in fp32)
NT = D // P   # 8 e-tiles / d-tiles / s-tiles
NB = 8     # batches == cores

VW = DK + 1  # 65: V columns per head incl. ones column
LAG = 3      # score/exp units emitted ahead of their attn@V consumer


def _emit(tc, io, phases="ASCVE"):
    nc = tc.nc

    const = tc.alloc_tile_pool(name="const", bufs=1)
    persist = tc.alloc_tile_pool(name="persist", bufs=1)
    stream = tc.alloc_tile_pool(name="stream", bufs=1)

    # --- constants -------------------------------------------------------
    # All const DMAs go through the Activation engine's DGE so the SP DGE can
    # start streaming phase-A x/w tiles immediately (HWDGE descriptor
    # processing is ~625ns each and serializes per engine queue).
    # per-partition bias columns: column t holds bias[t*128:(t+1)*128]
    bq_sb = const.tile([P, NT], F32, tag="bq", name="bq_sb")
    nc.scalar.dma_start(bq_sb[:], io["bqs"].ap().rearrange("(t p) -> p t", p=P))
    bk_sb = const.tile([P, NT], F32, tag="bk", name="bk_sb")
    nc.scalar.dma_start(bk_sb[:], io["bk"].ap().rearrange("(t p) -> p t", p=P))
    # row-broadcast bias tiles [128, D] for biases added along the free dim
    bv_bc = const.tile([P, D], F32, tag="bv", name="bv_bc")
    nc.scalar.dma_start(
        bv_bc[:].unsqueeze(1), io["bv"].ap().unsqueeze(0).partition_broadcast(P)
    )
    bo_bc = const.tile([P, D], F32, tag="bo", name="bo_bc")
    nc.scalar.dma_start(
        bo_bc[:].unsqueeze(1), io["bo"].ap().unsqueeze(0).partition_broadcast(P)
    )


    # --- persistent SBUF tensors (all bf16) ------------------------------
    QT = [persist.tile([P, S], BF16, tag=f"qt{t}", name=f"qt{t}") for t in range(NT)]
    KT = [persist.tile([P, S], BF16, tag=f"kt{t}", name=f"kt{t}") for t in range(NT)]
    # V with a ones column appended per head: [s, 16*65]
    V = [persist.tile([P, H * VW], BF16, tag=f"v{t}", name=f"v{t}") for t in range(NT)]
    OT = [persist.tile([P, S], BF16, tag=f"ot{t}", name=f"ot{t}") for t in range(NT)]
    # SBUF-resident xvT / wvT blocks for the by-strip V projection
    XV = [persist.tile([P, S], BF16, tag=f"xv{t}", name=f"xv{t}") for t in range(NT)]
    WV = [persist.tile([P, D], BF16, tag=f"wv{t}", name=f"wv{t}") for t in range(NT)]

    # ones columns of V (column 64 of each head's 65-wide group)
    for st in range(NT):
        v_view = V[st][:].rearrange("p (h k) -> p h k", k=VW)
        nc.scalar.dma_start(
            v_view[:, :, DK:VW].unsqueeze(1),
            io["onesw"].ap().unsqueeze(1).unsqueeze(0).partition_broadcast(P),
        )

    # Two PSUM pools, 2 slots x [128, 1024] (2 banks) each = all 8 banks.
    # Pool A: av/bc tiles in the unit loop; pool B: score tiles. Projections
    # draw 2 slots from each.
    poolA = tc.alloc_tile_pool(name="poolA", bufs=2, space="PSUM")
    poolB = tc.alloc_tile_pool(name="poolB", bufs=2, space="PSUM")

    def psum8(pfx):
        pairs = [
            (poolA if s < 2 else poolB).tile(
                [P, 2 * QC], F32, tag="u", name=f"{pfx}_{s}"
            )
            for s in range(NT // 2)
        ]
        return [pairs[t // 2][:, (t % 2) * QC : (t % 2 + 1) * QC] for t in range(NT)]

    # --- phase A: QT / KT projections (transposed, bias per-partition) ---
    # The c0 halves of Q and K are computed FIRST: score groups g0/g1 of the
    # qi=0 units only need QT cols 0:512 and KT k-blocks 0..3, so their
    # exps (the scalar engine is the structural bottleneck, ~1.9us/tile on
    # HW) start while the c1 halves are still projecting. c1 runs in two
    # 2-slot passes on pool A only, leaving pool B free for score tiles.
    dsb = tc.alloc_tile_pool(name="dsb", bufs=1)
    ats = {}

    def emit_scores_half(p, qi, half):
        qs = slice(qi * QC, (qi + 1) * QC)
        groups = ats.setdefault((p, qi), [])
        for g in ((0, 1) if half == 0 else (2, 3)):
            sce = poolB.tile([P, 2 * QC], F32, tag="u", name=f"sce{p}_{qi}_{g}")
            sco = poolB.tile([P, 2 * QC], F32, tag="u", name=f"sco{p}_{qi}_{g}")
            for j in range(2):
                kb = 2 * g + j
                ksl = slice(kb * P, (kb + 1) * P)
                nc.tensor.matmul(
                    sce[:, j * QC : (j + 1) * QC],
                    lhsT=KT[p][0:64, ksl],
                    rhs=QT[p][0:64, qs],
                    start=True,
                    stop=True,
                )
                nc.tensor.matmul(
                    sco[:, j * QC : (j + 1) * QC],
                    lhsT=KT[p][64:128, ksl],
                    rhs=QT[p][64:128, qs],
                    start=True,
                    stop=True,
                    tile_position=(64, 0),
                )
            at_dt = F32R if "F" in phases else BF16
            at_bufs = 14 if "F" in phases else 30
            ae = dsb.tile([P, 2 * QC], at_dt, tag="at", bufs=at_bufs,
                          name=f"ae{p}_{qi}_{g}")
            nc.scalar.activation(ae[:], sce[:], EXP)
            ao = dsb.tile([P, 2 * QC], at_dt, tag="at", bufs=at_bufs,
                          name=f"ao{p}_{qi}_{g}")
            nc.scalar.activation(ao[:], sco[:], EXP)
            groups.append((ae, ao))

    def emit_scores(p, qi):
        emit_scores_half(p, qi, 0)
        emit_scores_half(p, qi, 1)

    def a_c0(dst, wname, xname, bias):
        w_ap = io[wname].ap()
        x_ap = io[xname].ap()
        ps = psum8(f"ps_{wname}0")
        for d in range(NT):
            xt = stream.tile([P, QC], BF16, tag="xc", bufs=3,
                             name=f"x_{xname}0_{d}")
            nc.sync.dma_start(xt[:], x_ap[d * P : (d + 1) * P, 0:QC])
            wt = stream.tile([P, D], BF16, tag="big", bufs=8,
                             name=f"w_{wname}0_{d}")
            nc.sync.dma_start(wt[:], w_ap[d * P : (d + 1) * P, :])
            for t in range(NT):
                nc.tensor.matmul(
                    ps[t][:],
                    lhsT=wt[:, t * P : (t + 1) * P],
                    rhs=xt[:],
                    start=(d == 0),
                    stop=(d == NT - 1),
                )
        for t in range(NT):
            nc.vector.tensor_scalar_add(
                dst[t][:, 0:QC], ps[t][:], bias[:, t : t + 1]
            )

    def a_c1(dst, wname, xname, bias):
        w_ap = io[wname].ap()
        x_ap = io[xname].ap()
        for half in range(2):
            tiles = [
                poolA.tile([P, 2 * QC], F32, tag="u", name=f"pc1_{wname}{half}_{s}")
                for s in range(2)
            ]
            ps4 = [tiles[k // 2][:, (k % 2) * QC : (k % 2 + 1) * QC]
                   for k in range(4)]
            for d in range(NT):
                xt = stream.tile([P, QC], BF16, tag="xc", bufs=3,
                                 name=f"x_{xname}1{half}_{d}")
                nc.sync.dma_start(xt[:], x_ap[d * P : (d + 1) * P, QC : 2 * QC])
                wt = stream.tile([P, D], BF16, tag="big", bufs=8,
                                 name=f"w_{wname}1{half}_{d}")
                nc.sync.dma_start(wt[:], w_ap[d * P : (d + 1) * P, :])
                for k in range(4):
                    t = half * 4 + k
                    nc.tensor.matmul(
                        ps4[k],
                        lhsT=wt[:, t * P : (t + 1) * P],
                        rhs=xt[:],
                        start=(d == 0),
                        stop=(d == NT - 1),
                    )
            for k in range(4):
                t = half * 4 + k
                nc.vector.tensor_scalar_add(
                    dst[t][:, QC : 2 * QC], ps4[k][:], bias[:, t : t + 1]
                )

    units = [(p, 0) for p in range(NT)] + [(p, 1) for p in range(NT)]
    pipelined = "S" in phases and "V" in phases and "A" in phases
    N_EARLY = 5   # early qi=0 g0/g1 half-units (4 exp tiles each, cap 30)
    AT_CAP = 28

    if "A" in phases:
        a_c0(QT, "wqT", "xqT", bq_sb)
        a_c0(KT, "wkT", "xkT", bk_sb)
        if pipelined:
            for p in range(N_EARLY):
                emit_scores_half(p, 0, 0)
        a_c1(QT, "wqT", "xqT", bq_sb)
        if pipelined:
            for p in (5, 6):
                emit_scores_half(p, 0, 0)
        a_c1(KT, "wkT", "xkT", bk_sb)

    # prefetch the V-projection operands (phase C hides under the exp stretch)
    if "C" in phases:
        xv_ap = io["xvT"].ap()
        wv_ap = io["wvT"].ap()
        for d in range(NT):
            nc.sync.dma_start(XV[d][:], xv_ap[d * P : (d + 1) * P, :])
            nc.sync.dma_start(WV[d][:], wv_ap[d * P : (d + 1) * P, :])

    # chunk scheduler state: pending chunks in strict consumption-need order
    if pipelined:
        pend = [(p, 0, 1) for p in range(N_EARLY)]
        pend += [(5, 0, 1), (6, 0, 1), (7, 0, 0), (7, 0, 1)]
        for p in range(NT):
            pend += [(p, 1, 0), (p, 1, 1)]
        state = {"cur": 0, "inflight": 4 * (N_EARLY + 2)}

        def emit_chunk():
            p, qi, h = pend[state["cur"]]
            emit_scores_half(p, qi, h)
            state["cur"] += 1
            state["inflight"] += 4

        def top_up():
            while state["cur"] < len(pend) and state["inflight"] + 4 <= AT_CAP:
                emit_chunk()

        top_up()  # keep ACT fed through phase C
    elif "S" in phases:
        for p, qi in units:
            emit_scores(p, qi)

    # --- phase C: V projection by s-strip (1 pool-A slot per strip) ------
    # scores for unit LAG are emitted mid-C so the scalar engine stays fed
    # with exp work through the end of C (its 3 early units only cover ~25us
    # of the ~27us projection).
    for st in range(NT) if "C" in phases else ():
        vp = poolA.tile([P, 2 * QC], F32, tag="u", name=f"vps{st}")
        for d in range(NT):
            for c in range(2):
                nc.tensor.matmul(
                    vp[:, c * QC : (c + 1) * QC],
                    lhsT=XV[d][:, st * P : (st + 1) * P],
                    rhs=WV[d][:, c * QC : (c + 1) * QC],
                    start=(d == 0),
                    stop=(d == NT - 1),
                )
        v_out = V[st][:].rearrange("p (h k) -> p h k", k=VW)[:, :, 0:DK]
        ps_v = vp[:].rearrange("p (h k) -> p h k", k=DK)
        bv_v = bv_bc[:].rearrange("p (h k) -> p h k", k=DK)
        nc.vector.tensor_add(v_out, ps_v, bv_v)

    # --- unit loop: attn@V, normalize; scores for unit i+LAG in between --
    for i, (p, qi) in enumerate(units) if "V" in phases else ():
        he, ho = 2 * p, 2 * p + 1
        qs = slice(qi * QC, (qi + 1) * QC)
        while len(ats.get((p, qi), [])) < 4:  # force-complete this unit
            emit_chunk()
        groups = ats.pop((p, qi))

        av = poolA.tile([P, 2 * QC], F32, tag="u", name=f"av{p}_{qi}")
        ave = av[:, 0:QC]
        avo = av[:, QC : 2 * QC]
        for g in range(4):
            ae, ao = groups[g]
            for j in range(2):
                kb = 2 * g + j
                nc.tensor.matmul(
                    ave[0:VW, :],
                    lhsT=V[kb][:, he * VW : (he + 1) * VW],
                    rhs=ae[:, j * QC : (j + 1) * QC],
                    start=(kb == 0),
                    stop=(kb == NT - 1),
                )
                nc.tensor.matmul(
                    avo[0:VW, :],
                    lhsT=V[kb][:, ho * VW : (ho + 1) * VW],
                    rhs=ao[:, j * QC : (j + 1) * QC],
                    start=(kb == 0),
                    stop=(kb == NT - 1),
                )

        # one wide reciprocal of the sumexp row (partition 64, both parities)
        rec = dsb.tile([P, 2 * QC], F32, tag="rec", bufs=2, name=f"rec{p}_{qi}")
        nc.vector.reciprocal(rec[64:65, :], av[64:65, :])

        # keep PE busy with upcoming units' scores while DVE runs reciprocal
        state["inflight"] -= 8
        top_up()

        # broadcast the reciprocal row down to 64 partitions via a DRAM
        # bounce (SBUF->DRAM->SBUF partition-broadcast load, the proven
        # bias-load pattern; SP DGE is idle here and this keeps PE/PSUM out
        # of the normalize chain)
        scr = io["recscr"].ap()[i % 2]
        nc.sync.dma_start(scr, rec[64:65, :])
        rbc = dsb.tile([DK, 2 * QC], F32, tag="rbc", bufs=2, name=f"rbc{p}_{qi}")
        nc.sync.dma_start(
            rbc[:].unsqueeze(1), scr.unsqueeze(0).partition_broadcast(DK)
        )
        nc.vector.tensor_mul(OT[p][0:64, qs], ave[0:64, :], rbc[:, 0:QC])
        tmpo = dsb.tile([64, QC], BF16, tag="tmp", bufs=2, name=f"tmpo{p}_{qi}")
        nc.vector.tensor_mul(tmpo[:], avo[0:64, :], rbc[:, QC : 2 * QC])
        nc.sync.dma_start(OT[p][64:128, qs], tmpo[:])

    # --- phase E: output projection out[s, f] = OT.T @ woT + bo ----------
    out_ap = io["out"].ap()
    wo_ap = io["woT"].ap()
    if "E" not in phases:
        # bench-only: drain something comparable to E's output traffic
        srcs = OT if "V" in phases else QT
        for t in range(NT):
            nc.sync.dma_start(out_ap[t * P : (t + 1) * P, :], srcs[t][:, 0:S])
    for half in range(2) if "E" in phases else ():
        sts = list(range(4 * half, 4 * half + 4))
        tiles = [
            (poolA if k < 2 else poolB).tile(
                [P, 2 * QC], F32, tag="u", name=f"eo{half}_{k}"
            )
            for k in range(4)
        ]
        for e in range(NT):
            wt = stream.tile([P, D], BF16, tag="big", bufs=8, name=f"w_o{half}_{e}")
            nc.sync.dma_start(wt[:], wo_ap[e * P : (e + 1) * P, :])
            for k, st in enumerate(sts):
                lhs = OT[e][:, st * P : (st + 1) * P]
                nc.tensor.matmul(
                    tiles[k][:, 0:QC], lhsT=lhs, rhs=wt[:, 0:QC],
                    start=(e == 0), stop=(e == NT - 1),
                )
                nc.tensor.matmul(
                    tiles[k][:, QC : 2 * QC], lhsT=lhs, rhs=wt[:, QC : 2 * QC],
                    start=(e == 0), stop=(e == NT - 1),
                )
        for k, st in enumerate(sts):
            for c in range(2):
                fs = slice(c * QC, (c + 1) * QC)
                ob = stream.tile([P, QC], BF16, tag="ob", bufs=2,
                                 name=f"ob{half}_{st}_{c}")
                nc.vector.tensor_add(ob[:], tiles[k][:, fs], bo_bc[:, fs])
                nc.sync.dma_start(out_ap[st * P : (st + 1) * P, fs], ob[:])

    dsb.release()
    poolB.release()
    poolA.release()
    stream.release()
    persist.release()
    const.release()

def build_nc(repeats=1, phases="ASCVE"):
    nc = bacc.Bacc(
        "TRN2",
        target_bir_lowering=False,
        debug=False,
        enable_asserts=False,
        num_devices=NB,
    )
    io = {}
    for name in ("xqT", "xkT", "xvT"):
        io[name] = nc.dram_tensor(name, [D, S], BF16, kind="ExternalInput")
    for name in ("wqT", "wkT", "wvT", "woT"):
        io[name] = nc.dram_tensor(name, [D, D], BF16, kind="ExternalInput")
    for name in ("bqs", "bk", "bv", "bo"):
        io[name] = nc.dram_tensor(name, [D], F32, kind="ExternalInput")
    io["onesw"] = nc.dram_tensor("onesw", [H], BF16, kind="ExternalInput")
    io["out"] = nc.dram_tensor("out", [S, D], BF16, kind="ExternalOutput")
    io["recscr"] = nc.dram_tensor("recscr", [2, 2 * QC], F32, kind="Internal")

    with tile.TileContext(nc) as tc:
        for _ in range(repeats):
            _emit(tc, io, phases)
    nc.compile()
    return nc


_CACHE = {}


def get_nc():
    if "nc" not in _CACHE:
        _CACHE["nc"] = build_nc()
    return _CACHE["nc"]


def make_in_maps(query, key, value, wq, bq, wk, bk, wv, bv, wo, bo):
    f = np.float32
    # fold the 1/sqrt(DK) score scaling into the Q projection (exact: 1/8)
    wqT = (np.asarray(wq, f).T * f(0.125)).astype(NPBF16)
    bqs = np.asarray(bq, f) * f(0.125)
    wkT = np.asarray(wk, f).T.astype(NPBF16)
    wvT = np.asarray(wv, f).T.astype(NPBF16)
    woT = np.asarray(wo, f).T.astype(NPBF16)
    common = {
        "wqT": np.ascontiguousarray(wqT),
        "wkT": np.ascontiguousarray(wkT),
        "wvT": np.ascontiguousarray(wvT),
        "woT": np.ascontiguousarray(woT),
        "bqs": np.ascontiguousarray(bqs),
        "bk": np.ascontiguousarray(np.asarray(bk, f)),
        "bv": np.ascontiguousarray(np.asarray(bv, f)),
        "bo": np.ascontiguousarray(np.asarray(bo, f)),
        "onesw": np.ones(H, NPBF16),
    }
    q = np.asarray(query, f)
    k = np.asarray(key, f)
    v = np.asarray(value, f)
    in_maps = []
    for b in range(NB):
        in_maps.append(
            {
                "xqT": np.ascontiguousarray(q[b].T.astype(NPBF16)),
                "xkT": np.ascontiguousarray(k[b].T.astype(NPBF16)),
                "xvT": np.ascontiguousarray(v[b].T.astype(NPBF16)),
                **common,
            }
        )
    return in_maps


def kernel(
    query,
    key,
    value,
    inputs_attn_mask=None,  # all-ones per spec; masking is a no-op
    wq=None, bq=None, wk=None, bk=None, wv=None, bv=None, wo=None, bo=None,
    **_extra,
):
    nc = get_nc()
    in_maps = make_in_maps(query, key, value, wq, bq, wk, bk, wv, bv, wo, bo)
    res = run_bass_kernel_spmd(nc, in_maps, core_ids=list(range(NB)))
    out = np.stack(
        [np.asarray(res.results[b]["out"]).astype(np.float32) for b in range(NB)],
        axis=0,
    )
    return out
